# revision 1
# baseline (speedup 1.0000x reference)
"""Trainium2 Bass kernel for nn_BoxAwareAttention: full attention block
(QKV proj + bias, RoPE, scaled-dot-product attention with softmax, out proj).

Sharding over 8 NeuronCores: data-parallel over batch (2) x tensor-parallel
over heads (16 -> 4 per core).  Core c handles batch c//4, heads 4*(c%4)..+4.
Each core computes its partial projection output (contraction over its 256
channels); host sums the 4 partials per batch and adds the (linear) v-bias
contribution b_v @ w_proj.T as a constant row.

v6 schedule.  Hardware facts driving it: (1) a HAM power governor clips the
PE to half clock when it runs >~83% busy in 3.4us windows, with long
hysteresis, so both bursts and idles are expensive — the optimum is ~80%
PE duty everywhere; (2) DVE ops complete-serialize (~1.25us per full-width
op), so a q/k tile's RoPE chain (4 partition-shifted rotate-half reads
from PSUM + cos multiply, DVE-only: GpSimd cannot touch PSUM and SBUF
operands must share base partitions) is ~6us of serial DVE; (3) ACT is the
only exp engine: 128 exps x 1.11us = 142.5us is the attention-phase pole.

  - Prologue holds only what attention(qb0, h0) needs: kT pair-0 (second
    column half arrives mid-window), qT(block0, pair0), and all of V.
    xa DMAs are split by column half so the first projection starts ~2us
    after the DMA queue opens.
  - All other q/k tiles, and q-block-0's out-proj, are *fed* into the
    attention windows one or two matmuls per k-tile step — fine-grained
    enough to ride the ~250ns/kt PE slack without saturating a window.
    Their rope/evac finishers are held until after the window's norm is
    issued so the DVE queue stays [norm, then ropes].
  - PV trails QK/exp by 8 k-tiles and each window's last 8 PV matmuls +
    softmax normalization are flushed early in the NEXT window (carry),
    so exps never wait on the norm chain at window boundaries.
  - v natural [L, d] (bias applied on host) with an appended ones column:
    the softmax denominator falls out of the P@V matmul as row 64; plain
    DVE copy stages it to base partition 0 (custom-DVE ops mishandle
    nonzero AP bases), fast reciprocal, partition_broadcast on GpSimd.
  - q-block 1's out-proj ping-pongs PSUM between the proj pool and the
    then-idle score pool with ACT evac.  Output DMA'd as bf16.
"""

import os
import sys

for _p in ("/opt/trn_rl_repo", "/root/.axon_site/_ro/trn_rl_repo"):
    if os.path.isdir(_p) and _p not in sys.path:
        sys.path.insert(0, _p)

import numpy as np
import ml_dtypes

import concourse.bass as bass
import concourse.mybir as mybir
import concourse.tile as tile
from concourse import bacc
from concourse import bass_utils

BF16 = ml_dtypes.bfloat16
N_CORES = 8
B, L_FULL, C, H, D = 2, 2048, 1024, 16, 64
H_LOC = 4              # heads per core
M_LOC = H_LOC * D      # 256 output channels per core
KCH = 8                # qkv contraction chunks (1024 = 8*128)
TRAIL = 8              # PV lag behind QK/exp, in k-tiles
F32 = mybir.dt.float32
BF = mybir.dt.bfloat16
ADD = mybir.AluOpType.add
MULT = mybir.AluOpType.mult


def build_program(L=L_FULL, num_devices=N_CORES):
    """Build the per-core Bass program (SPMD: same program, per-core data)."""
    NB = min(1024, L)      # q/L block width (PSUM tile free size)
    nNB = L // NB          # number of blocks
    nKT = L // 128         # attention k-tiles / v L-tiles
    nMT = 2                # q/k M-tiles (2 heads of 64 each)
    nQT = L // 128         # proj q-tiles
    PW = min(512, NB)      # PV/norm sub-block width
    SPL = [bass.ds(s, PW) for s in range(0, NB, PW)]

    nc = bacc.Bacc("TRN2", target_bir_lowering=False, debug=False,
                   num_devices=num_devices)

    xa_d = nc.dram_tensor("xa", [KCH, 128, L], BF, kind="ExternalInput").ap()
    wq_d = nc.dram_tensor("wq", [128, KCH, M_LOC], BF, kind="ExternalInput").ap()
    wk_d = nc.dram_tensor("wk", [128, KCH, M_LOC], BF, kind="ExternalInput").ap()
    wv_d = nc.dram_tensor("wv", [128, KCH, M_LOC], BF, kind="ExternalInput").ap()
    wp_d = nc.dram_tensor("wp", [128, 2, C], BF, kind="ExternalInput").ap()
    bb_d = nc.dram_tensor("bb", [128, 4], F32, kind="ExternalInput").ap()
    cs_d = nc.dram_tensor("cs", [128, 2, L], BF, kind="ExternalInput").ap()
    o_d = nc.dram_tensor("o", [nQT, 128, C], BF, kind="ExternalOutput").ap()

    with tile.TileContext(nc) as tc:
        with (
            tc.tile_pool(name="const", bufs=1) as cpool,
            tc.tile_pool(name="rope", bufs=3) as rpool,
            tc.tile_pool(name="pt", bufs=11) as ptpool,
            tc.tile_pool(name="norm", bufs=3) as npool,
            tc.tile_pool(name="outs", bufs=3) as opool,
            tc.tile_pool(name="ps_big", bufs=2, space="PSUM") as ps_big,
            tc.tile_pool(name="ps_proj", bufs=1, space="PSUM") as ps_proj,
            tc.tile_pool(name="ps_o", bufs=2, space="PSUM") as ps_o,
        ):
            xa = [cpool.tile([128, L], BF, tag=f"xa{c}", name=f"xa{c}")
                  for c in range(KCH)]
            wq = cpool.tile([128, KCH, M_LOC], BF, tag="wq")
            wk = cpool.tile([128, KCH, M_LOC], BF, tag="wk")
            wv = cpool.tile([128, KCH, M_LOC], BF, tag="wv")
            wp = cpool.tile([128, 2, C], BF, tag="wp")
            bb = cpool.tile([128, 4], F32, tag="bb")
            cs = cpool.tile([128, 2, L], BF, tag="cs")
            qT = [cpool.tile([128, L], BF, tag=f"qT{m}", name=f"qT{m}") for m in range(nMT)]
            kT = [cpool.tile([128, L], BF, tag=f"kT{m}", name=f"kT{m}") for m in range(nMT)]
            oT = [cpool.tile([128, L], BF, tag=f"oT{m}", name=f"oT{m}") for m in range(nMT)]
            v_aug = cpool.tile([128, nKT, H_LOC, 65], BF, tag="vaug")

            # input DMAs ordered by first use; xa split by column half so
            # the first K projection isn't gated on the full activation load
            h0c = bass.ds(0, NB)
            h1c = bass.ds(NB, NB)
            nc.sync.dma_start(wk[:], wk_d[:])
            nc.sync.dma_start(xa[0][:, h0c], xa_d[0][:, h0c])
            nc.sync.dma_start(wq[:], wq_d[:])
            nc.sync.dma_start(cs[:], cs_d[:])
            nc.sync.dma_start(bb[:], bb_d[:])
            for c in range(1, KCH):
                nc.sync.dma_start(xa[c][:, h0c], xa_d[c][:, h0c])
            nc.sync.dma_start(wv[:], wv_d[:])
            for c in range(KCH):
                nc.sync.dma_start(xa[c][:, h1c], xa_d[c][:, h1c])
            nc.sync.dma_start(wp[:], wp_d[:])
            nc.vector.memset(v_aug[:, :, :, 64:65], 1.0)

            cos_s = cs[:, 0, :]
            sinx_s = cs[:, 1, :]

            # ---- q/k projection tile: matmuls + bias + RoPE ----
            qk_idx = [0]
            def qk_mms(pq, w_s, mt, ls):
                for cc in range(KCH):
                    for sp in SPL:
                        nc.tensor.matmul(
                            pq[:, sp], w_s[:, cc, bass.ts(mt, 128)],
                            xa[cc][:, ls][:, sp],
                            start=(cc == 0), stop=(cc == KCH - 1))

            def qk_fin(pq, w_s, dstT, mt, ls, tag):
                # bias in-place on PSUM; rotate-half = partition-shifted
                # PSUM reads (DVE only)
                ti = 0 if w_s is wq else 1
                nc.scalar.activation(
                    pq[:], pq[:],
                    mybir.ActivationFunctionType.Identity,
                    bias=bb[:, ti * 2 + mt:ti * 2 + mt + 1])
                rot = rpool.tile([128, NB], BF, tag="rot", name=f"rot{tag}")
                for do, so in ((0, 32), (32, 0), (64, 96), (96, 64)):
                    nc.vector.tensor_tensor(
                        rot[do:do + 32, :], pq[so:so + 32, :],
                        sinx_s[do:do + 32, ls], MULT)
                tcos = rpool.tile([128, NB], BF, tag="tcos", name=f"tcos{tag}")
                nc.vector.tensor_tensor(tcos[:], pq[:], cos_s[:, ls], MULT)
                nc.gpsimd.tensor_tensor(dstT[mt][:, ls], tcos[:], rot[:], ADD)

            def qk_tile(w_s, dstT, mt, lb, pool=None):
                # prologue tiles rotate the score pool by default, keeping
                # the proj slot free for the first feeder matmuls
                ls = bass.ds(lb * NB, NB)
                pq = (pool or ps_big).tile(
                    [128, NB], F32, tag="pp" if pool is ps_proj else "big")
                qk_idx[0] += 1
                qk_mms(pq, w_s, mt, ls)
                qk_fin(pq, w_s, dstT, mt, ls, f"p{mt}{lb}")

            # ---- one V tile: natural [L, d], no bias, ACT evac, ps_o ----
            def v_tile(lt):
                pv = ps_o.tile([128, M_LOC], F32, tag="po", name=f"pv{lt}")
                for cc in range(KCH):
                    nc.tensor.matmul(
                        pv[:], xa[cc][:, bass.ts(lt, 128)], wv[:, cc, :],
                        start=(cc == 0), stop=(cc == KCH - 1))
                nc.scalar.copy(
                    v_aug[:, lt, :, 0:64],
                    pv[:].rearrange("p (h d) -> p h d", h=H_LOC))

            # ---- prologue: K(pair0, first half), Q(block0, pair0), V;
            # K(pair0, second half) right after the second xa halves land
            # rope chains in consumer order: the first exp needs only
            # K(pair0, cols 0-1023) and Q(block0, pair0); K's second half
            # isn't consumed until kt8, ~9us after the first exp
            qk_tile(wk, kT, 0, 0)
            for lt in range(4):
                v_tile(lt)
            qk_tile(wq, qT, 0, 0)
            for lt in range(4, 8):
                v_tile(lt)
            qk_tile(wk, kT, 0, 1)
            for lt in range(8, nKT):
                v_tile(lt)
            qk_tile(wq, qT, 1, 0, pool=ps_proj)

            # ---- out-proj pieces ----
            def oproj_mms(pp, qt):
                for cc in range(2):
                    for nn in range(2):
                        nc.tensor.matmul(
                            pp[:, bass.ts(nn, C // 2)],
                            oT[cc][:, bass.ts(qt, 128)],
                            wp[:, cc, bass.ts(nn, C // 2)],
                            start=(cc == 0), stop=(cc == 1))

            def oproj_evac(pp, qt, evac):
                ost = opool.tile([128, C], BF, tag="ost", name=f"ost{qt}")
                if evac == "act":
                    nc.scalar.copy(ost[:], pp[:])
                else:
                    nc.vector.tensor_copy(ost[:], pp[:])
                nc.sync.dma_start(o_d[qt], ost[:])

            # ---- feeder: deferred work, 1-2 matmuls per k-tile step; DVE
            # finishers (rope / evac) held until the window's norm is issued
            feed_q = []

            def feed_qk(w_s, dstT, mt, lb):
                ls = bass.ds(lb * NB, NB)
                st8 = {}
                def alloc():
                    if "pq" not in st8:
                        st8["pq"] = ps_proj.tile(
                            [128, NB], F32, tag="pp", name=f"fq{mt}{lb}")
                    return st8["pq"]
                for cc in range(KCH):
                    for sp in SPL:
                        def mm(cc=cc, sp=sp):
                            nc.tensor.matmul(
                                alloc()[:, sp], w_s[:, cc, bass.ts(mt, 128)],
                                xa[cc][:, ls][:, sp],
                                start=(cc == 0), stop=(cc == KCH - 1))
                        feed_q.append({"f": mm, "fin": False})
                def fin():
                    qk_fin(st8["pq"], w_s, dstT, mt, ls, f"f{mt}{lb}")
                feed_q.append({"f": fin, "fin": True})

            def feed_oproj(qt):
                st8 = {}
                def alloc():
                    if "pp" not in st8:
                        st8["pp"] = ps_proj.tile(
                            [128, C], F32, tag="pp", name=f"fo{qt}")
                    return st8["pp"]
                for cc in range(2):
                    for nn in range(2):
                        def mm(cc=cc, nn=nn):
                            nc.tensor.matmul(
                                alloc()[:, bass.ts(nn, C // 2)],
                                oT[cc][:, bass.ts(qt, 128)],
                                wp[:, cc, bass.ts(nn, C // 2)],
                                start=(cc == 0), stop=(cc == 1))
                        feed_q.append({"f": mm, "fin": False})
                def fin():
                    oproj_evac(st8["pp"], qt, "dve")
                feed_q.append({"f": fin, "fin": True})

            def pump(kt, n):
                for _ in range(n):
                    if not feed_q:
                        return
                    if feed_q[0]["fin"] and kt < 6:
                        return
                    feed_q.pop(0)["f"]()

            # per-window deferred work.  qT(pair1, block0) and kT(pair1)
            # must be roped before window (0,2); qT(block1) before qb1.
            feeders = {
                (1, 0): [(feed_oproj, (qt,)) for qt in range(4)],
                (1, 1): [(feed_oproj, (qt,)) for qt in range(4, 8)],
            }
            # pair-1 K tiles and block-1 Q tiles injected at kt10: their
            # ropes clear the DVE well before the next window's carried
            # norm, and each lands before its first consumer (pair-1 at
            # h2, block-1 at qb1)
            inject_mid = {
                (0, 0): (wk, kT, 1, 0),
                (0, 1): (wk, kT, 1, 1),
                (0, 2): (wq, qT, 0, 1),
                (0, 3): (wq, qT, 1, 1),
            }

            def norm_head(pos_c, mt_c, pr_c, qs_c):
                for si, sp in enumerate(SPL):
                    po = pos_c[si]
                    # softmax denominator: row 64 (ones column of v_aug);
                    # staged to base partition 0 by a plain DVE copy
                    dn = npool.tile([1, PW], F32, tag="dn")
                    nc.vector.tensor_copy(dn[:], po[64:65, :])
                    rc = npool.tile([1, PW], F32, tag="rc")
                    nc.vector.reciprocal_approx_fast(rc[:], dn[:])
                    rb = npool.tile([64, PW], F32, tag="rb")
                    nc.gpsimd.partition_broadcast(rb[:], rc[:], channels=64)
                    nc.vector.tensor_tensor(
                        oT[mt_c][pr_c, qs_c][:, sp], po[0:64, :], rb[:], MULT)

            def pv_issue(pos_w, h_w, pts_w, ktc, last):
                for si, sp in enumerate(SPL):
                    nc.tensor.matmul(
                        pos_w[si][:], v_aug[:, ktc, h_w, :],
                        pts_w[ktc][:, sp],
                        start=(ktc == 0), stop=(ktc == nKT - 1 and last))

            # ---- attention: exps never pause; PV trails by TRAIL k-tiles
            # and each window's trailing PVs + norm flush early in the next
            carry = None
            for qb in range(nNB):
                qs = bass.ds(qb * NB, NB)
                for h in range(H_LOC):
                    mt, hh = divmod(h, 2)
                    pr = slice(64 * hh, 64 * hh + 64)
                    for fn, args in feeders.get((qb, h), []):
                        fn(*args)
                    pos = [ps_o.tile([65, PW], F32, tag="po",
                                     name=f"po{qb}{h}{si}")
                           for si in range(len(SPL))]
                    pts = {}
                    for kt in range(nKT):
                        st = ps_big.tile([128, NB], F32, tag="big")
                        for sp in SPL:
                            nc.tensor.matmul(
                                st[:, sp], kT[mt][pr, bass.ts(kt, 128)],
                                qT[mt][pr, qs][:, sp],
                                start=True, stop=True)
                        pt = ptpool.tile([128, NB], BF, tag="pt")
                        nc.scalar.activation(
                            pt[:], st[:], mybir.ActivationFunctionType.Exp,
                            scale=float(D) ** -0.5)
                        pts[kt] = pt
                        if carry is not None and 1 <= kt <= 5:
                            c_pos, c_h, c_pts, c_mt, c_pr, c_qs = carry
                            if kt <= 4:
                                for ktc in (nKT - TRAIL + 2 * kt - 2,
                                            nKT - TRAIL + 2 * kt - 1):
                                    pv_issue(c_pos, c_h, c_pts, ktc, True)
                            else:
                                norm_head(c_pos, c_mt, c_pr, c_qs)
                                carry = None
                        if kt >= TRAIL:
                            pv_issue(pos, h, pts, kt - TRAIL, False)
                        if kt == 10 and (qb, h) in inject_mid:
                            w_s_, d_, mt_, lb_ = inject_mid[(qb, h)]
                            qk_tile(w_s_, d_, mt_, lb_, pool=ps_proj)
                        # qb1 feeders read qb0's oT, whose last head's norm
                        # is carry-flushed at kt5 — pump only after that
                        if qb == 0 or kt >= 6:
                            pump(kt, 2)
                    carry = (pos, h, pts, mt, pr, qs)
            # final window's trailing PVs + norm
            c_pos, c_h, c_pts, c_mt, c_pr, c_qs = carry
            for ktc in range(nKT - TRAIL, nKT):
                pv_issue(c_pos, c_h, c_pts, ktc, True)
            norm_head(c_pos, c_mt, c_pr, c_qs)
            while feed_q:
                feed_q.pop(0)["f"]()

            # tail: q-block 1's out-proj, ping-pong proj/score PSUM pools
            for j in range(NB // 128):
                qt = NB // 128 + j
                pp = (ps_proj if j % 2 == 0 else ps_big).tile(
                    [128, C], F32, tag="pp" if j % 2 == 0 else "big",
                    name=f"tp{qt}")
                oproj_mms(pp, qt)
                oproj_evac(pp, qt, "act")

    nc.compile()
    return nc


_CACHE = {}


def _get_program(L=L_FULL):
    if L not in _CACHE:
        _CACHE[L] = build_program(L)
    return _CACHE[L]


def make_core_inputs(x, w_qkv, b_qkv, w_proj, cos, sin, L=L_FULL):
    """Host-side shard/transpose/pad/cast. Returns in_maps for the 8 cores."""
    x = np.asarray(x, np.float32)
    w_qkv = np.asarray(w_qkv, np.float32)
    b_qkv = np.asarray(b_qkv, np.float32)
    w_proj = np.asarray(w_proj, np.float32)
    cos = np.asarray(cos, np.float32)
    sin = np.asarray(sin, np.float32)

    # replicated rope tables: [128, L] (2 heads stacked), sign folded into sin
    cT = cos.T.astype(np.float32)                      # [64, L]
    sT = sin.T.astype(np.float32)
    cosT = np.concatenate([cT, cT], 0)                 # [128, L]
    sx = np.concatenate([-sT[0:32], sT[32:64]], 0)
    sinX = np.concatenate([sx, sx], 0)
    cs = np.ascontiguousarray(
        np.stack([cosT, sinX], axis=1)).astype(BF16)   # [128, 2, L]

    def pack_k(mat_t):
        # mat_t: [C, M] (already transposed) -> [128, KCH, M] bf16
        Cdim, M = mat_t.shape
        assert Cdim == KCH * 128
        return np.ascontiguousarray(
            mat_t.reshape(KCH, 128, M).transpose(1, 0, 2)).astype(BF16)

    in_maps = []
    for c in range(N_CORES):
        b, hg = divmod(c, 4)
        h0 = H_LOC * hg
        r = slice(h0 * D, (h0 + H_LOC) * D)            # head-channel rows
        # per-partition q/k biases for the [d, L] layout (cols: q-mt0,
        # q-mt1, k-mt0, k-mt1); v bias is applied on the host
        bq, bk = b_qkv[r], b_qkv[C:][r]
        bqk = np.stack([bq[:128], bq[128:], bk[:128], bk[128:]], 1)
        in_maps.append({
            "xa": np.ascontiguousarray(
                x[b].T.reshape(KCH, 128, L)).astype(BF16),
            "wq": pack_k(w_qkv[r].T),
            "wk": pack_k(w_qkv[C:][r].T),
            "wv": pack_k(w_qkv[2 * C:][r].T),
            "wp": np.ascontiguousarray(
                w_proj[:, r].T.reshape(2, 128, C).transpose(1, 0, 2)).astype(BF16),
            "bb": np.ascontiguousarray(bqk, np.float32),
            "cs": cs,
        })
    return in_maps


def kernel(x, w_qkv, b_qkv, w_proj, cos, sin, mask=None, trace=False):
    nc = _get_program()
    in_maps = make_core_inputs(x, w_qkv, b_qkv, w_proj, cos, sin)
    res = bass_utils.run_bass_kernel_spmd(
        nc, in_maps, core_ids=list(range(N_CORES)), trace=trace)
    # v-bias enters the output linearly: + b_v @ w_proj.T on every row
    bias_row = (np.asarray(b_qkv, np.float32)[2 * C:]
                @ np.asarray(w_proj, np.float32).T)
    out = np.zeros((B, L_FULL, C), np.float32)
    for c in range(N_CORES):
        out[c // 4] += res.results[c]["o"].astype(np.float32).reshape(L_FULL, C)
    out += bias_row[None, None, :]
    if trace:
        kernel.last_results = res
    return out



# revision 17
# speedup vs baseline: 1.0787x; 1.0787x over previous
"""Trainium2 Bass kernel for nn_BoxAwareAttention: full attention block
(QKV proj + bias, RoPE, scaled-dot-product attention with softmax, out proj).

Sharding over 8 NeuronCores: data-parallel over batch (2) x tensor-parallel
over heads (16 -> 4 per core).  Core c handles batch c//4, heads 4*(c%4)..+4.
Each core computes its partial projection output (contraction over its 256
channels); host sums the 4 partials per batch and adds the (linear) v-bias
contribution b_v @ w_proj.T as a constant row.

v6 schedule.  Hardware facts driving it: (1) a HAM power governor clips the
PE to half clock when it runs >~83% busy in 3.4us windows, with long
hysteresis, so both bursts and idles are expensive — the optimum is ~80%
PE duty everywhere; (2) DVE ops complete-serialize (~1.25us per full-width
op), so a q/k tile's RoPE chain (4 partition-shifted rotate-half reads
from PSUM + cos multiply, DVE-only: GpSimd cannot touch PSUM and SBUF
operands must share base partitions) is ~6us of serial DVE; (3) ACT is the
only exp engine: 128 exps x 1.11us = 142.5us is the attention-phase pole.

  - Prologue holds only what attention(qb0, h0..1) strictly needs before
    its kt8: kT pair-0 and qT(block0, pair0) (three rope chains — the DVE
    serial floor), plus all of V.  xa and the rope tables are DMA'd in
    halves so only ~3.7MB gates the first K projection.  The window loop
    starts right after k01; everything else — qT(mt1,b0), kT pair-1,
    block-1 q tiles, qb0's out-proj — is *fed* into the windows.
  - Feeders run 2-3 matmuls per k-tile step through the single ps_proj
    slot; 8 nop slots after each rope finisher keep the next feeder's
    matmuls from blocking the in-order PE on the slot's rope reads.
    Finishers are held only in (1,0) (oT completeness); elsewhere a fed
    rope jumping a carried norm just delays pos recycling 2 windows out.
  - PV trails QK/exp by 8 k-tiles and each window's last 8 PV matmuls +
    softmax normalization are flushed early in the NEXT window (carry),
    so exps never wait on the norm chain at window boundaries.
  - v natural [L, d] (bias applied on host) with an appended ones column:
    the softmax denominator falls out of the P@V matmul as row 64; plain
    DVE copy stages it to base partition 0 (custom-DVE ops mishandle
    nonzero AP bases), fast reciprocal, partition_broadcast on GpSimd.
  - q-block 1's out-proj ping-pongs PSUM between the proj pool and the
    then-idle score pool with ACT evac.  Output DMA'd as bf16.
"""

import os
import sys

for _p in ("/opt/trn_rl_repo", "/root/.axon_site/_ro/trn_rl_repo"):
    if os.path.isdir(_p) and _p not in sys.path:
        sys.path.insert(0, _p)

import numpy as np
import ml_dtypes

import concourse.bass as bass
import concourse.mybir as mybir
import concourse.tile as tile
from concourse import bacc
from concourse import bass_utils

BF16 = ml_dtypes.bfloat16
N_CORES = 8
B, L_FULL, C, H, D = 2, 2048, 1024, 16, 64
H_LOC = 4              # heads per core
M_LOC = H_LOC * D      # 256 output channels per core
KCH = 8                # qkv contraction chunks (1024 = 8*128)
TRAIL = 10             # PV lag behind QK/exp, in k-tiles
F32 = mybir.dt.float32
BF = mybir.dt.bfloat16
ADD = mybir.AluOpType.add
MULT = mybir.AluOpType.mult


def build_program(L=L_FULL, num_devices=N_CORES):
    """Build the per-core Bass program (SPMD: same program, per-core data)."""
    NB = min(1024, L)      # q/L block width (PSUM tile free size)
    nNB = L // NB          # number of blocks
    nKT = L // 128         # attention k-tiles / v L-tiles
    nMT = 2                # q/k M-tiles (2 heads of 64 each)
    nQT = L // 128         # proj q-tiles
    PW = min(512, NB)      # PV/norm sub-block width
    SPL = [bass.ds(s, PW) for s in range(0, NB, PW)]

    nc = bacc.Bacc("TRN2", target_bir_lowering=False, debug=False,
                   num_devices=num_devices)

    xa_d = nc.dram_tensor("xa", [KCH, 128, L], BF, kind="ExternalInput").ap()
    wq_d = nc.dram_tensor("wq", [128, KCH, M_LOC], BF, kind="ExternalInput").ap()
    wk_d = nc.dram_tensor("wk", [128, KCH, M_LOC], BF, kind="ExternalInput").ap()
    wv_d = nc.dram_tensor("wv", [128, KCH, M_LOC], BF, kind="ExternalInput").ap()
    wp_d = nc.dram_tensor("wp", [128, 2, C], BF, kind="ExternalInput").ap()
    bb_d = nc.dram_tensor("bb", [128, 4], F32, kind="ExternalInput").ap()
    cs_d = nc.dram_tensor("cs", [128, 2, L], BF, kind="ExternalInput").ap()
    o_d = nc.dram_tensor("o", [nQT, 128, C], BF, kind="ExternalOutput").ap()

    with tile.TileContext(nc) as tc:
        with (
            tc.tile_pool(name="const", bufs=1) as cpool,
            tc.tile_pool(name="rope", bufs=3) as rpool,
            tc.tile_pool(name="pt", bufs=12) as ptpool,
            tc.tile_pool(name="norm", bufs=3) as npool,
            tc.tile_pool(name="outs", bufs=3) as opool,
            tc.tile_pool(name="ps_big", bufs=2, space="PSUM") as ps_big,
            tc.tile_pool(name="ps_proj", bufs=1, space="PSUM") as ps_proj,
            tc.tile_pool(name="ps_o", bufs=2, space="PSUM") as ps_o,
        ):
            xa = [cpool.tile([128, L], BF, tag=f"xa{c}", name=f"xa{c}")
                  for c in range(KCH)]
            wq = cpool.tile([128, KCH, M_LOC], BF, tag="wq")
            wk = cpool.tile([128, KCH, M_LOC], BF, tag="wk")
            wv = cpool.tile([128, KCH, M_LOC], BF, tag="wv")
            wp = cpool.tile([128, 2, C], BF, tag="wp")
            bb = cpool.tile([128, 4], F32, tag="bb")
            cs = cpool.tile([128, 2, L], BF, tag="cs")
            qT = [cpool.tile([128, L], BF, tag=f"qT{m}", name=f"qT{m}") for m in range(nMT)]
            kT = [cpool.tile([128, L], BF, tag=f"kT{m}", name=f"kT{m}") for m in range(nMT)]
            oT = [cpool.tile([128, L], BF, tag=f"oT{m}", name=f"oT{m}") for m in range(nMT)]
            v_aug = cpool.tile([128, nKT, H_LOC, 65], BF, tag="vaug")

            # input DMAs ordered by first use; xa split by column half so
            # the first K projection isn't gated on the full activation load.
            # cs split by L-half too: only the lb0 half gates the first ropes.
            h0c = bass.ds(0, NB)
            h1c = bass.ds(NB, NB)
            nc.sync.dma_start(wk[:], wk_d[:])
            nc.sync.dma_start(xa[0][:, h0c], xa_d[0][:, h0c])
            nc.sync.dma_start(wq[:], wq_d[:])
            nc.sync.dma_start(cs[:, :, h0c], cs_d[:, :, h0c])
            nc.sync.dma_start(bb[:], bb_d[:])
            for c in range(1, KCH):
                nc.sync.dma_start(xa[c][:, h0c], xa_d[c][:, h0c])
            nc.sync.dma_start(wv[:], wv_d[:])
            for c in range(KCH):
                nc.sync.dma_start(xa[c][:, h1c], xa_d[c][:, h1c])
            nc.sync.dma_start(cs[:, :, h1c], cs_d[:, :, h1c])
            nc.sync.dma_start(wp[:], wp_d[:])
            nc.vector.memset(v_aug[:, :, :, 64:65], 1.0)

            cos_s = cs[:, 0, :]
            sinx_s = cs[:, 1, :]

            # ---- q/k projection tile: matmuls + bias + RoPE ----
            qk_idx = [0]
            def qk_mms(pq, w_s, mt, ls):
                for cc in range(KCH):
                    for sp in SPL:
                        nc.tensor.matmul(
                            pq[:, sp], w_s[:, cc, bass.ts(mt, 128)],
                            xa[cc][:, ls][:, sp],
                            start=(cc == 0), stop=(cc == KCH - 1))

            def qk_fin(pq, w_s, dstT, mt, ls, tag):
                # bias in-place on PSUM; rotate-half = partition-shifted
                # PSUM reads (DVE only)
                ti = 0 if w_s is wq else 1
                nc.scalar.activation(
                    pq[:], pq[:],
                    mybir.ActivationFunctionType.Identity,
                    bias=bb[:, ti * 2 + mt:ti * 2 + mt + 1])
                rot = rpool.tile([128, NB], BF, tag="rot", name=f"rot{tag}")
                for do, so in ((0, 32), (32, 0), (64, 96), (96, 64)):
                    nc.vector.tensor_tensor(
                        rot[do:do + 32, :], pq[so:so + 32, :],
                        sinx_s[do:do + 32, ls], MULT)
                tcos = rpool.tile([128, NB], BF, tag="tcos", name=f"tcos{tag}")
                nc.vector.tensor_tensor(tcos[:], pq[:], cos_s[:, ls], MULT)
                nc.gpsimd.tensor_tensor(dstT[mt][:, ls], tcos[:], rot[:], ADD)

            def qk_tile(w_s, dstT, mt, lb, pool=None):
                # prologue tiles rotate the score pool by default, keeping
                # the proj slot free for the first feeder matmuls
                ls = bass.ds(lb * NB, NB)
                pq = (pool or ps_big).tile(
                    [128, NB], F32, tag="pp" if pool is ps_proj else "big")
                qk_idx[0] += 1
                qk_mms(pq, w_s, mt, ls)
                qk_fin(pq, w_s, dstT, mt, ls, f"p{mt}{lb}")

            # ---- one V tile: natural [L, d], no bias, ACT evac, ps_o ----
            def v_tile(lt):
                pv = ps_o.tile([128, M_LOC], F32, tag="po", name=f"pv{lt}")
                for cc in range(KCH):
                    nc.tensor.matmul(
                        pv[:], xa[cc][:, bass.ts(lt, 128)], wv[:, cc, :],
                        start=(cc == 0), stop=(cc == KCH - 1))
                nc.scalar.copy(
                    v_aug[:, lt, :, 0:64],
                    pv[:].rearrange("p (h d) -> p h d", h=H_LOC))

            # ---- prologue: only what window (0,0) needs before its kt8:
            # kT(pair0), qT(block0, pair0), and all of V.  Everything else
            # (qT mt1, kT pair1, block-1 q tiles, out-proj) drips through
            # the feed queue during the windows.  Rope chains serialize on
            # DVE, so the window loop starts right after k01's matmuls.
            qk_tile(wk, kT, 0, 0)
            for lt in range(4):
                v_tile(lt)
            qk_tile(wq, qT, 0, 0)
            for lt in range(4, 8):
                v_tile(lt)
            qk_tile(wk, kT, 0, 1)
            for lt in range(8, nKT):
                v_tile(lt)

            # ---- out-proj pieces ----
            def oproj_mms(pp, qt):
                for cc in range(2):
                    for nn in range(2):
                        nc.tensor.matmul(
                            pp[:, bass.ts(nn, C // 2)],
                            oT[cc][:, bass.ts(qt, 128)],
                            wp[:, cc, bass.ts(nn, C // 2)],
                            start=(cc == 0), stop=(cc == 1))

            def oproj_evac(pp, qt, evac):
                ost = opool.tile([128, C], BF, tag="ost", name=f"ost{qt}")
                if evac == "act":
                    nc.scalar.copy(ost[:], pp[:])
                else:
                    nc.vector.tensor_copy(ost[:], pp[:])
                nc.sync.dma_start(o_d[qt], ost[:])

            # ---- feeder: deferred work, 1-2 matmuls per k-tile step; DVE
            # finishers (rope / evac) held until the window's norm is issued
            feed_q = []

            def feed_qk(w_s, dstT, mt, lb):
                ls = bass.ds(lb * NB, NB)
                st8 = {}
                def alloc():
                    if "pq" not in st8:
                        st8["pq"] = ps_proj.tile(
                            [128, NB], F32, tag="pp", name=f"fq{mt}{lb}")
                    return st8["pq"]
                for cc in range(KCH):
                    for sp in SPL:
                        def mm(cc=cc, sp=sp):
                            nc.tensor.matmul(
                                alloc()[:, sp], w_s[:, cc, bass.ts(mt, 128)],
                                xa[cc][:, ls][:, sp],
                                start=(cc == 0), stop=(cc == KCH - 1))
                        feed_q.append({"f": mm, "fin": False})
                def fin():
                    qk_fin(st8["pq"], w_s, dstT, mt, ls, f"f{mt}{lb}")
                feed_q.append({"f": fin, "fin": True})
                # the rope chain reads the ps_proj tile for ~5 DVE ops
                # (~4 kt); hold the next feeder off that long
                feed_nops(8)

            def feed_nops(n):
                # spacing entries: burn pump slots so the next feeder's
                # matmuls aren't popped before the ps_proj slot is free
                # (a blocked matmul stalls the in-order PE stream)
                for _ in range(n):
                    feed_q.append({"f": lambda: None, "fin": False})

            def feed_oproj(qt):
                st8 = {}
                def alloc():
                    if "pp" not in st8:
                        st8["pp"] = ps_proj.tile(
                            [128, C], F32, tag="pp", name=f"fo{qt}")
                    return st8["pp"]
                for cc in range(2):
                    for nn in range(2):
                        def mm(cc=cc, nn=nn):
                            nc.tensor.matmul(
                                alloc()[:, bass.ts(nn, C // 2)],
                                oT[cc][:, bass.ts(qt, 128)],
                                wp[:, cc, bass.ts(nn, C // 2)],
                                start=(cc == 0), stop=(cc == 1))
                        feed_q.append({"f": mm, "fin": False})
                def fin():
                    oproj_evac(st8["pp"], qt, "dve")
                feed_q.append({"f": fin, "fin": True})
                feed_nops(2)

            def pump(kt, n, block_fins=False):
                for _ in range(n):
                    if not feed_q:
                        return
                    if feed_q[0]["fin"] and block_fins:
                        return
                    feed_q.pop(0)["f"]()

            # per-window deferred work, all through the paced feed queue
            # (ps_proj ring): qT(mt1,b0) + kT(pair1) feed during (0,0)/(0,1)
            # (consumed from (0,2)), block-1 Q tiles during (0,2)/(0,3)
            # (consumed from qb1), qb0's out-proj during (1,0)/(1,1).
            feeders = {
                (0, 0): [(feed_qk, (wq, qT, 1, 0)),
                         (feed_qk, (wk, kT, 1, 0))],
                (0, 1): [(feed_qk, (wk, kT, 1, 1))],
                (0, 2): [(feed_qk, (wq, qT, 0, 1))],
                (0, 3): [(feed_qk, (wq, qT, 1, 1))],
                (1, 0): [(feed_oproj, (qt,)) for qt in range(4)],
                (1, 1): [(feed_oproj, (qt,)) for qt in range(4, 8)],
            }

            def norm_head_sp(pos_c, mt_c, pr_c, qs_c, si):
                sp = SPL[si]
                po = pos_c[si]
                # softmax denominator: row 64 (ones column of v_aug);
                # staged to base partition 0 by a plain DVE copy
                dn = npool.tile([1, PW], F32, tag="dn")
                nc.vector.tensor_copy(dn[:], po[64:65, :])
                rc = npool.tile([1, PW], F32, tag="rc")
                nc.vector.reciprocal_approx_fast(rc[:], dn[:])
                rb = npool.tile([64, PW], F32, tag="rb")
                nc.gpsimd.partition_broadcast(rb[:], rc[:], channels=64)
                nc.vector.tensor_tensor(
                    oT[mt_c][pr_c, qs_c][:, sp], po[0:64, :], rb[:], MULT)

            def norm_head(pos_c, mt_c, pr_c, qs_c):
                for si in range(len(SPL)):
                    norm_head_sp(pos_c, mt_c, pr_c, qs_c, si)

            def pv_issue(pos_w, h_w, pts_w, ktc, last):
                for si, sp in enumerate(SPL):
                    nc.tensor.matmul(
                        pos_w[si][:], v_aug[:, ktc, h_w, :],
                        pts_w[ktc][:, sp],
                        start=(ktc == 0), stop=(ktc == nKT - 1 and last))

            # ---- attention: exps never pause; PV trails by TRAIL k-tiles
            # and each window's trailing PVs + norm flush early in the next
            carry = None
            for qb in range(nNB):
                qs = bass.ds(qb * NB, NB)
                for h in range(H_LOC):
                    mt, hh = divmod(h, 2)
                    pr = slice(64 * hh, 64 * hh + 64)
                    for fn, args in feeders.get((qb, h), []):
                        fn(*args)
                    pos = [ps_o.tile([65, PW], F32, tag="po",
                                     name=f"po{qb}{h}{si}")
                           for si in range(len(SPL))]
                    pts = {}
                    for kt in range(nKT):
                        st = ps_big.tile([128, NB], F32, tag="big")
                        for sp in SPL:
                            nc.tensor.matmul(
                                st[:, sp], kT[mt][pr, bass.ts(kt, 128)],
                                qT[mt][pr, qs][:, sp],
                                start=True, stop=True)
                        pt = ptpool.tile([128, NB], BF, tag="pt")
                        nc.scalar.activation(
                            pt[:], st[:], mybir.ActivationFunctionType.Exp,
                            scale=float(D) ** -0.5)
                        pts[kt] = pt
                        if carry is not None and 1 <= kt <= 6:
                            c_pos, c_h, c_pts, c_mt, c_pr, c_qs = carry
                            if kt <= 5:
                                for ktc in (nKT - TRAIL + 2 * kt - 2,
                                            nKT - TRAIL + 2 * kt - 1):
                                    pv_issue(c_pos, c_h, c_pts, ktc, True)
                            else:
                                norm_head(c_pos, c_mt, c_pr, c_qs)
                                carry = None
                        if kt >= TRAIL:
                            pv_issue(pos, h, pts, kt - TRAIL, False)
                        # (1,0) feeders read qb0's oT, whose last head's
                        # norm is carry-flushed at kt6 — no pumping before
                        # that (a blocked matmul stalls the in-order PE).
                        # While a carry is pending, hold rope/evac fins so
                        # the carried norm's DVE ops aren't queued behind a
                        # 5-op rope chain (pos recycling gates next PV).
                        if (qb, h) != (1, 0) or kt >= 7:
                            pump(kt, 3 if (qb == 0 and h <= 1) else 2,
                                 block_fins=(carry is not None))
                    carry = (pos, h, pts, mt, pr, qs)
            # final window: flush trailing PVs si-major so norm(sp0) issues
            # while si1's PVs still stream on the PE; interleave the two
            # norm halves with the out-proj tiles they gate (qt8-11 read
            # only sp0 columns of oT, qt12-15 only sp1), and spread the
            # evacuations over ACT and DVE so the tail isn't one-engine
            # paced.
            c_pos, c_h, c_pts, c_mt, c_pr, c_qs = carry
            for si, sp in enumerate(SPL):
                for ktc in range(nKT - TRAIL, nKT):
                    nc.tensor.matmul(
                        c_pos[si][:], v_aug[:, ktc, c_h, :],
                        c_pts[ktc][:, sp],
                        start=False, stop=(ktc == nKT - 1))
                if si == 0:
                    norm_head_sp(c_pos, c_mt, c_pr, c_qs, 0)
            while feed_q:
                feed_q.pop(0)["f"]()

            # tail: q-block 1's out-proj, ping-pong proj/score PSUM pools
            for j in range(NB // 128):
                qt = NB // 128 + j
                pp = (ps_proj if j % 2 == 0 else ps_big).tile(
                    [128, C], F32, tag="pp" if j % 2 == 0 else "big",
                    name=f"tp{qt}")
                oproj_mms(pp, qt)
                oproj_evac(pp, qt, "act" if j < 4 else ("dve" if j % 2 else "act"))
                if j == 3:
                    norm_head_sp(c_pos, c_mt, c_pr, c_qs, 1)

    nc.compile()
    return nc


_CACHE = {}


def _get_program(L=L_FULL):
    if L not in _CACHE:
        _CACHE[L] = build_program(L)
    return _CACHE[L]


def make_core_inputs(x, w_qkv, b_qkv, w_proj, cos, sin, L=L_FULL):
    """Host-side shard/transpose/pad/cast. Returns in_maps for the 8 cores."""
    x = np.asarray(x, np.float32)
    w_qkv = np.asarray(w_qkv, np.float32)
    b_qkv = np.asarray(b_qkv, np.float32)
    w_proj = np.asarray(w_proj, np.float32)
    cos = np.asarray(cos, np.float32)
    sin = np.asarray(sin, np.float32)

    # replicated rope tables: [128, L] (2 heads stacked), sign folded into sin
    cT = cos.T.astype(np.float32)                      # [64, L]
    sT = sin.T.astype(np.float32)
    cosT = np.concatenate([cT, cT], 0)                 # [128, L]
    sx = np.concatenate([-sT[0:32], sT[32:64]], 0)
    sinX = np.concatenate([sx, sx], 0)
    cs = np.ascontiguousarray(
        np.stack([cosT, sinX], axis=1)).astype(BF16)   # [128, 2, L]

    def pack_k(mat_t):
        # mat_t: [C, M] (already transposed) -> [128, KCH, M] bf16
        Cdim, M = mat_t.shape
        assert Cdim == KCH * 128
        return np.ascontiguousarray(
            mat_t.reshape(KCH, 128, M).transpose(1, 0, 2)).astype(BF16)

    in_maps = []
    for c in range(N_CORES):
        b, hg = divmod(c, 4)
        h0 = H_LOC * hg
        r = slice(h0 * D, (h0 + H_LOC) * D)            # head-channel rows
        # per-partition q/k biases for the [d, L] layout (cols: q-mt0,
        # q-mt1, k-mt0, k-mt1); v bias is applied on the host
        bq, bk = b_qkv[r], b_qkv[C:][r]
        bqk = np.stack([bq[:128], bq[128:], bk[:128], bk[128:]], 1)
        in_maps.append({
            "xa": np.ascontiguousarray(
                x[b].T.reshape(KCH, 128, L)).astype(BF16),
            "wq": pack_k(w_qkv[r].T),
            "wk": pack_k(w_qkv[C:][r].T),
            "wv": pack_k(w_qkv[2 * C:][r].T),
            "wp": np.ascontiguousarray(
                w_proj[:, r].T.reshape(2, 128, C).transpose(1, 0, 2)).astype(BF16),
            "bb": np.ascontiguousarray(bqk, np.float32),
            "cs": cs,
        })
    return in_maps


def kernel(x, w_qkv, b_qkv, w_proj, cos, sin, mask=None, trace=False):
    nc = _get_program()
    in_maps = make_core_inputs(x, w_qkv, b_qkv, w_proj, cos, sin)
    res = bass_utils.run_bass_kernel_spmd(
        nc, in_maps, core_ids=list(range(N_CORES)), trace=trace)
    # v-bias enters the output linearly: + b_v @ w_proj.T on every row
    bias_row = (np.asarray(b_qkv, np.float32)[2 * C:]
                @ np.asarray(w_proj, np.float32).T)
    out = np.zeros((B, L_FULL, C), np.float32)
    for c in range(N_CORES):
        out[c // 4] += res.results[c]["o"].astype(np.float32).reshape(L_FULL, C)
    out += bias_row[None, None, :]
    if trace:
        kernel.last_results = res
    return out



# revision 26
# speedup vs baseline: 1.0994x; 1.0192x over previous
"""Trainium2 Bass kernel for nn_BoxAwareAttention: full attention block
(QKV proj + bias, RoPE, scaled-dot-product attention with softmax, out proj).

Sharding over 8 NeuronCores: data-parallel over batch (2) x tensor-parallel
over heads (16 -> 4 per core).  Core c handles batch c//4, heads 4*(c%4)..+4.
Each core computes its partial projection output (contraction over its 256
channels); host sums the 4 partials per batch and adds the (linear) v-bias
contribution b_v @ w_proj.T as a constant row.

v6 schedule.  Hardware facts driving it: (1) a HAM power governor clips the
PE to half clock when it runs >~83% busy in 3.4us windows, with long
hysteresis, so both bursts and idles are expensive — the optimum is ~80%
PE duty everywhere; (2) DVE ops complete-serialize (~1.25us per full-width
op), so a q/k tile's RoPE chain (4 partition-shifted rotate-half reads
from PSUM + cos multiply, DVE-only: GpSimd cannot touch PSUM and SBUF
operands must share base partitions) is ~6us of serial DVE; (3) ACT is the
only exp engine: 128 exps x 1.11us = 142.5us is the attention-phase pole.

  - Prologue holds only what attention(qb0, h0..1) strictly needs before
    its kt8: kT pair-0 and qT(block0, pair0) (three rope chains — the DVE
    serial floor), plus all of V.  xa and the rope tables are DMA'd in
    halves so only ~3.7MB gates the first K projection.  The window loop
    starts right after k01; everything else — qT(mt1,b0), kT pair-1,
    block-1 q tiles, qb0's out-proj — is *fed* into the windows.
  - Feeders run 2-3 matmuls per k-tile step through the single ps_proj
    slot; 8 nop slots after each rope finisher keep the next feeder's
    matmuls from blocking the in-order PE on the slot's rope reads.
    Finishers are held only in (1,0) (oT completeness); elsewhere a fed
    rope jumping a carried norm just delays pos recycling 2 windows out.
  - PV trails QK/exp by 8 k-tiles and each window's last 8 PV matmuls +
    softmax normalization are flushed early in the NEXT window (carry),
    so exps never wait on the norm chain at window boundaries.
  - v natural [L, d] (bias applied on host) with an appended ones column:
    the softmax denominator falls out of the P@V matmul as row 64; plain
    DVE copy stages it to base partition 0 (custom-DVE ops mishandle
    nonzero AP bases), fast reciprocal, partition_broadcast on GpSimd.
  - q-block 1's out-proj ping-pongs PSUM between the proj pool and the
    then-idle score pool with ACT evac.  Output DMA'd as bf16.
"""

import os
import sys

for _p in ("/opt/trn_rl_repo", "/root/.axon_site/_ro/trn_rl_repo"):
    if os.path.isdir(_p) and _p not in sys.path:
        sys.path.insert(0, _p)

import numpy as np
import ml_dtypes

import concourse.bass as bass
import concourse.mybir as mybir
import concourse.tile as tile
from concourse import bacc
from concourse import bass_utils

BF16 = ml_dtypes.bfloat16
N_CORES = 8
B, L_FULL, C, H, D = 2, 2048, 1024, 16, 64
H_LOC = 4              # heads per core
M_LOC = H_LOC * D      # 256 output channels per core
KCH = 8                # qkv contraction chunks (1024 = 8*128)
TRAIL = 10             # PV lag behind QK/exp, in k-tiles
F32 = mybir.dt.float32
BF = mybir.dt.bfloat16
ADD = mybir.AluOpType.add
MULT = mybir.AluOpType.mult


def build_program(L=L_FULL, num_devices=N_CORES):
    """Build the per-core Bass program (SPMD: same program, per-core data)."""
    NB = min(1024, L)      # q/L block width (PSUM tile free size)
    nNB = L // NB          # number of blocks
    nKT = L // 128         # attention k-tiles / v L-tiles
    nMT = 2                # q/k M-tiles (2 heads of 64 each)
    nQT = L // 128         # proj q-tiles
    PW = min(512, NB)      # PV/norm sub-block width
    SPL = [bass.ds(s, PW) for s in range(0, NB, PW)]

    nc = bacc.Bacc("TRN2", target_bir_lowering=False, debug=False,
                   num_devices=num_devices)

    xa_d = nc.dram_tensor("xa", [KCH, 128, L], BF, kind="ExternalInput").ap()
    wq_d = nc.dram_tensor("wq", [128, KCH, M_LOC], BF, kind="ExternalInput").ap()
    wk_d = nc.dram_tensor("wk", [128, KCH, M_LOC], BF, kind="ExternalInput").ap()
    wv_d = nc.dram_tensor("wv", [128, KCH, M_LOC], BF, kind="ExternalInput").ap()
    wp_d = nc.dram_tensor("wp", [128, 2, C], BF, kind="ExternalInput").ap()
    bb_d = nc.dram_tensor("bb", [128, 4], F32, kind="ExternalInput").ap()
    cs_d = nc.dram_tensor("cs", [128, 2, L], BF, kind="ExternalInput").ap()
    o_d = nc.dram_tensor("o", [nQT, 128, C], BF, kind="ExternalOutput").ap()

    with tile.TileContext(nc) as tc:
        with (
            tc.tile_pool(name="const", bufs=1) as cpool,
            tc.tile_pool(name="rope", bufs=3) as rpool,
            tc.tile_pool(name="pt", bufs=12) as ptpool,
            tc.tile_pool(name="norm", bufs=3) as npool,
            tc.tile_pool(name="outs", bufs=3) as opool,
            tc.tile_pool(name="ps_big", bufs=2, space="PSUM") as ps_big,
            tc.tile_pool(name="ps_proj", bufs=1, space="PSUM") as ps_proj,
            tc.tile_pool(name="ps_o", bufs=2, space="PSUM") as ps_o,
        ):
            xa = [cpool.tile([128, L], BF, tag=f"xa{c}", name=f"xa{c}")
                  for c in range(KCH)]
            wq = cpool.tile([128, KCH, M_LOC], BF, tag="wq")
            wk = cpool.tile([128, KCH, M_LOC], BF, tag="wk")
            wv = cpool.tile([128, KCH, M_LOC], BF, tag="wv")
            wp = cpool.tile([128, 2, C], BF, tag="wp")
            bb = cpool.tile([128, 4], F32, tag="bb")
            cs = cpool.tile([128, 2, L], BF, tag="cs")
            qT = [cpool.tile([128, L], BF, tag=f"qT{m}", name=f"qT{m}") for m in range(nMT)]
            kT = [cpool.tile([128, L], BF, tag=f"kT{m}", name=f"kT{m}") for m in range(nMT)]
            oT = [cpool.tile([128, L], BF, tag=f"oT{m}", name=f"oT{m}") for m in range(nMT)]
            v_aug = cpool.tile([128, nKT, H_LOC, 65], BF, tag="vaug")

            # input DMAs ordered by first use; xa split by column half so
            # the first K projection isn't gated on the full activation load.
            # cs split by L-half too: only the lb0 half gates the first ropes.
            h0c = bass.ds(0, NB)
            h1c = bass.ds(NB, NB)
            nc.sync.dma_start(wk[:], wk_d[:])
            nc.sync.dma_start(xa[0][:, h0c], xa_d[0][:, h0c])
            nc.sync.dma_start(cs[:, :, h0c], cs_d[:, :, h0c])
            nc.sync.dma_start(bb[:], bb_d[:])
            nc.sync.dma_start(wq[:], wq_d[:])
            for c in range(1, KCH):
                nc.sync.dma_start(xa[c][:, h0c], xa_d[c][:, h0c])
            nc.sync.dma_start(wv[:], wv_d[:])
            for c in range(KCH):
                nc.sync.dma_start(xa[c][:, h1c], xa_d[c][:, h1c])
            nc.sync.dma_start(cs[:, :, h1c], cs_d[:, :, h1c])
            nc.sync.dma_start(wp[:], wp_d[:])
            nc.vector.memset(v_aug[:, :, :, 64:65], 1.0)

            cos_s = cs[:, 0, :]
            sinx_s = cs[:, 1, :]

            # ---- q/k projection tile: matmuls + bias + RoPE ----
            qk_idx = [0]
            def qk_mms(pq, w_s, mt, ls):
                for cc in range(KCH):
                    for sp in SPL:
                        nc.tensor.matmul(
                            pq[:, sp], w_s[:, cc, bass.ts(mt, 128)],
                            xa[cc][:, ls][:, sp],
                            start=(cc == 0), stop=(cc == KCH - 1))

            def qk_fin(pq, w_s, dstT, mt, ls, tag):
                # bias in-place on PSUM; rotate-half = partition-shifted
                # PSUM reads (DVE only)
                ti = 0 if w_s is wq else 1
                nc.scalar.activation(
                    pq[:], pq[:],
                    mybir.ActivationFunctionType.Identity,
                    bias=bb[:, ti * 2 + mt:ti * 2 + mt + 1])
                rot = rpool.tile([128, NB], BF, tag="rot", name=f"rot{tag}")
                for do, so in ((0, 32), (32, 0), (64, 96), (96, 64)):
                    nc.vector.tensor_tensor(
                        rot[do:do + 32, :], pq[so:so + 32, :],
                        sinx_s[do:do + 32, ls], MULT)
                tcos = rpool.tile([128, NB], BF, tag="tcos", name=f"tcos{tag}")
                nc.vector.tensor_tensor(tcos[:], pq[:], cos_s[:, ls], MULT)
                nc.gpsimd.tensor_tensor(dstT[mt][:, ls], tcos[:], rot[:], ADD)

            def qk_tile(w_s, dstT, mt, lb, pool=None):
                # prologue tiles rotate the score pool by default, keeping
                # the proj slot free for the first feeder matmuls
                ls = bass.ds(lb * NB, NB)
                pq = (pool or ps_big).tile(
                    [128, NB], F32, tag="pp" if pool is ps_proj else "big")
                qk_idx[0] += 1
                qk_mms(pq, w_s, mt, ls)
                qk_fin(pq, w_s, dstT, mt, ls, f"p{mt}{lb}")

            # ---- one V tile: natural [L, d], no bias, ACT evac, ps_o ----
            def v_tile(lt):
                pv = ps_o.tile([128, M_LOC], F32, tag="po", name=f"pv{lt}")
                for cc in range(KCH):
                    nc.tensor.matmul(
                        pv[:], xa[cc][:, bass.ts(lt, 128)], wv[:, cc, :],
                        start=(cc == 0), stop=(cc == KCH - 1))
                nc.scalar.copy(
                    v_aug[:, lt, :, 0:64],
                    pv[:].rearrange("p (h d) -> p h d", h=H_LOC))

            # ---- out-proj pieces ----
            def oproj_mms(pp, qt):
                for cc in range(2):
                    for nn in range(2):
                        nc.tensor.matmul(
                            pp[:, bass.ts(nn, C // 2)],
                            oT[cc][:, bass.ts(qt, 128)],
                            wp[:, cc, bass.ts(nn, C // 2)],
                            start=(cc == 0), stop=(cc == 1))

            def oproj_evac(pp, qt, evac):
                ost = opool.tile([128, C], BF, tag="ost", name=f"ost{qt}")
                if evac == "act":
                    nc.scalar.copy(ost[:], pp[:])
                else:
                    nc.vector.tensor_copy(ost[:], pp[:])
                nc.sync.dma_start(o_d[qt], ost[:])

            # ---- feeder: deferred work, 1-2 matmuls per k-tile step; DVE
            # finishers (rope / evac) held until the window's norm is issued
            feed_q = []

            def feed_qk(w_s, dstT, mt, lb, tail_nops=12):
                ls = bass.ds(lb * NB, NB)
                st8 = {}
                def alloc():
                    if "pq" not in st8:
                        st8["pq"] = ps_proj.tile(
                            [128, NB], F32, tag="pp", name=f"fq{mt}{lb}")
                    return st8["pq"]
                for cc in range(KCH):
                    for sp in SPL:
                        def mm(cc=cc, sp=sp):
                            nc.tensor.matmul(
                                alloc()[:, sp], w_s[:, cc, bass.ts(mt, 128)],
                                xa[cc][:, ls][:, sp],
                                start=(cc == 0), stop=(cc == KCH - 1))
                        feed_q.append({"f": mm, "fin": False})
                def fin():
                    qk_fin(st8["pq"], w_s, dstT, mt, ls, f"f{mt}{lb}")
                feed_q.append({"f": fin, "fin": True})
                # the rope chain reads the ps_proj tile for ~5 DVE ops
                # (~5-6 kt); hold the next feeder off that long
                feed_nops(tail_nops)

            def feed_nops(n):
                # spacing entries: burn pump slots so the next feeder's
                # matmuls aren't popped before the ps_proj slot is free
                # (a blocked matmul stalls the in-order PE stream)
                for _ in range(n):
                    feed_q.append({"f": lambda: None, "fin": False})

            def feed_oproj(qt):
                st8 = {}
                def alloc():
                    if "pp" not in st8:
                        st8["pp"] = ps_proj.tile(
                            [128, C], F32, tag="pp", name=f"fo{qt}")
                    return st8["pp"]
                for cc in range(2):
                    for nn in range(2):
                        def mm(cc=cc, nn=nn):
                            nc.tensor.matmul(
                                alloc()[:, bass.ts(nn, C // 2)],
                                oT[cc][:, bass.ts(qt, 128)],
                                wp[:, cc, bass.ts(nn, C // 2)],
                                start=(cc == 0), stop=(cc == 1))
                        feed_q.append({"f": mm, "fin": False})
                def fin():
                    oproj_evac(st8["pp"], qt, "dve")
                feed_q.append({"f": fin, "fin": True})
                feed_nops(2)

            def pump(kt, n, block_fins=False):
                for _ in range(n):
                    if not feed_q:
                        return
                    if feed_q[0]["fin"] and block_fins:
                        return
                    feed_q.pop(0)["f"]()

            # ---- prologue: only what window (0,0) needs before its kt10:
            # kT(pair0), qT(block0, pair0), and all of V.  Everything else
            # flows through the feed queue: qT(mt1,b0) / kT(pair1) / the
            # block-1 q tiles chain one-at-a-time through the single
            # ps_proj slot (~13 kt each: 8 kt of matmuls + rope reads),
            # so the chain starts in the DMA-shadowed prologue (PE is
            # half idle there) via explicit pump calls, sized so only
            # q10's entries drain before the window loop.
            feed_qk(wq, qT, 1, 0, tail_nops=20)
            feed_qk(wk, kT, 1, 0, tail_nops=12)
            feed_qk(wk, kT, 1, 1, tail_nops=16)
            # k00, q00 and q10's fed matmuls all pace with the per-chunk
            # xa arrivals — they fill the DMA shadow; v and k01 follow.
            # The window loop starts right after v15, gated by q00's rope.
            qk_tile(wk, kT, 0, 0)
            qk_tile(wq, qT, 0, 0)
            pump(-1, 16)
            for lt in range(12):
                v_tile(lt)
            qk_tile(wk, kT, 0, 1)
            pump(-1, 8)
            for lt in range(12, nKT):
                v_tile(lt)
                pump(-1, 2)

            # remaining per-window feeder registration: q01 during (0,1)
            # (consumed from (1,0)), q11 during (0,2) (consumed (1,2)),
            # qb0's out-proj spread 2 tiles/window over (1,0)-(1,3).
            feeders = {
                (0, 1): [(feed_qk, (wq, qT, 0, 1))],
                (0, 2): [(feed_qk, (wq, qT, 1, 1))],
                (1, 0): [(feed_oproj, (qt,)) for qt in range(2)],
                (1, 1): [(feed_oproj, (qt,)) for qt in range(2, 4)],
                (1, 2): [(feed_oproj, (qt,)) for qt in range(4, 6)],
                (1, 3): [(feed_oproj, (qt,)) for qt in range(6, 8)],
            }

            def norm_head_sp(pos_c, mt_c, pr_c, qs_c, si):
                sp = SPL[si]
                po = pos_c[si]
                # softmax denominator: row 64 (ones column of v_aug);
                # staged to base partition 0 by a plain DVE copy
                dn = npool.tile([1, PW], F32, tag="dn")
                nc.vector.tensor_copy(dn[:], po[64:65, :])
                rc = npool.tile([1, PW], F32, tag="rc")
                nc.vector.reciprocal_approx_fast(rc[:], dn[:])
                rb = npool.tile([64, PW], F32, tag="rb")
                nc.gpsimd.partition_broadcast(rb[:], rc[:], channels=64)
                nc.vector.tensor_tensor(
                    oT[mt_c][pr_c, qs_c][:, sp], po[0:64, :], rb[:], MULT)

            def norm_head(pos_c, mt_c, pr_c, qs_c):
                for si in range(len(SPL)):
                    norm_head_sp(pos_c, mt_c, pr_c, qs_c, si)

            def pv_issue(pos_w, h_w, pts_w, ktc, last):
                for si, sp in enumerate(SPL):
                    nc.tensor.matmul(
                        pos_w[si][:], v_aug[:, ktc, h_w, :],
                        pts_w[ktc][:, sp],
                        start=(ktc == 0), stop=(ktc == nKT - 1 and last))

            # ---- attention: exps never pause; PV trails by TRAIL k-tiles
            # and each window's trailing PVs + norm flush early in the next
            carry = None
            for qb in range(nNB):
                qs = bass.ds(qb * NB, NB)
                for h in range(H_LOC):
                    mt, hh = divmod(h, 2)
                    pr = slice(64 * hh, 64 * hh + 64)
                    for fn, args in feeders.get((qb, h), []):
                        fn(*args)
                    pos = [ps_o.tile([65, PW], F32, tag="po",
                                     name=f"po{qb}{h}{si}")
                           for si in range(len(SPL))]
                    pts = {}
                    for kt in range(nKT):
                        st = ps_big.tile([128, NB], F32, tag="big")
                        for sp in SPL:
                            nc.tensor.matmul(
                                st[:, sp], kT[mt][pr, bass.ts(kt, 128)],
                                qT[mt][pr, qs][:, sp],
                                start=True, stop=True)
                        pt = ptpool.tile([128, NB], BF, tag="pt")
                        nc.scalar.activation(
                            pt[:], st[:], mybir.ActivationFunctionType.Exp,
                            scale=float(D) ** -0.5)
                        pts[kt] = pt
                        if carry is not None and 1 <= kt <= 6:
                            c_pos, c_h, c_pts, c_mt, c_pr, c_qs = carry
                            if kt <= 5:
                                for ktc in (nKT - TRAIL + 2 * kt - 2,
                                            nKT - TRAIL + 2 * kt - 1):
                                    pv_issue(c_pos, c_h, c_pts, ktc, True)
                            else:
                                norm_head(c_pos, c_mt, c_pr, c_qs)
                                carry = None
                        # last window trails less so its PVs mostly finish
                        # inside the window (shorter serial tail)
                        trail_w = 4 if (qb, h) == (nNB - 1, H_LOC - 1) else TRAIL
                        if kt >= trail_w:
                            pv_issue(pos, h, pts, kt - trail_w, False)
                        # (1,0) feeders read qb0's oT, whose last head's
                        # norm is carry-flushed at kt6 — no pumping before
                        # that (a blocked matmul stalls the in-order PE).
                        # While a carry is pending, hold rope/evac fins so
                        # the carried norm's DVE ops aren't queued behind a
                        # 5-op rope chain (pos recycling gates next PV).
                        if (qb, h) != (1, 0) or kt >= 7:
                            pump(kt, 2, block_fins=(carry is not None))
                    carry = (pos, h, pts, mt, pr, qs)
            # final window: flush trailing PVs si-major so norm(sp0) issues
            # while si1's PVs still stream on the PE; interleave the two
            # norm halves with the out-proj tiles they gate (qt8-11 read
            # only sp0 columns of oT, qt12-15 only sp1), and spread the
            # evacuations over ACT and DVE so the tail isn't one-engine
            # paced.
            c_pos, c_h, c_pts, c_mt, c_pr, c_qs = carry
            for si, sp in enumerate(SPL):
                for ktc in range(nKT - 4, nKT):
                    nc.tensor.matmul(
                        c_pos[si][:], v_aug[:, ktc, c_h, :],
                        c_pts[ktc][:, sp],
                        start=False, stop=(ktc == nKT - 1))
                if si == 0:
                    norm_head_sp(c_pos, c_mt, c_pr, c_qs, 0)
            while feed_q:
                feed_q.pop(0)["f"]()

            # tail: q-block 1's out-proj, ping-pong proj/score PSUM pools,
            # evacuations alternating ACT/DVE so neither engine paces it
            for j in range(NB // 128):
                qt = NB // 128 + j
                pp = (ps_proj if j % 2 == 0 else ps_big).tile(
                    [128, C], F32, tag="pp" if j % 2 == 0 else "big",
                    name=f"tp{qt}")
                oproj_mms(pp, qt)
                oproj_evac(pp, qt, "act" if j % 2 == 0 else "dve")
                if j == 3:
                    norm_head_sp(c_pos, c_mt, c_pr, c_qs, 1)

    nc.compile()
    return nc


_CACHE = {}


def _get_program(L=L_FULL):
    if L not in _CACHE:
        _CACHE[L] = build_program(L)
    return _CACHE[L]


def make_core_inputs(x, w_qkv, b_qkv, w_proj, cos, sin, L=L_FULL):
    """Host-side shard/transpose/pad/cast. Returns in_maps for the 8 cores."""
    x = np.asarray(x, np.float32)
    w_qkv = np.asarray(w_qkv, np.float32)
    b_qkv = np.asarray(b_qkv, np.float32)
    w_proj = np.asarray(w_proj, np.float32)
    cos = np.asarray(cos, np.float32)
    sin = np.asarray(sin, np.float32)

    # replicated rope tables: [128, L] (2 heads stacked), sign folded into sin
    cT = cos.T.astype(np.float32)                      # [64, L]
    sT = sin.T.astype(np.float32)
    cosT = np.concatenate([cT, cT], 0)                 # [128, L]
    sx = np.concatenate([-sT[0:32], sT[32:64]], 0)
    sinX = np.concatenate([sx, sx], 0)
    cs = np.ascontiguousarray(
        np.stack([cosT, sinX], axis=1)).astype(BF16)   # [128, 2, L]

    def pack_k(mat_t):
        # mat_t: [C, M] (already transposed) -> [128, KCH, M] bf16
        Cdim, M = mat_t.shape
        assert Cdim == KCH * 128
        return np.ascontiguousarray(
            mat_t.reshape(KCH, 128, M).transpose(1, 0, 2)).astype(BF16)

    in_maps = []
    for c in range(N_CORES):
        b, hg = divmod(c, 4)
        h0 = H_LOC * hg
        r = slice(h0 * D, (h0 + H_LOC) * D)            # head-channel rows
        # per-partition q/k biases for the [d, L] layout (cols: q-mt0,
        # q-mt1, k-mt0, k-mt1); v bias is applied on the host
        bq, bk = b_qkv[r], b_qkv[C:][r]
        bqk = np.stack([bq[:128], bq[128:], bk[:128], bk[128:]], 1)
        in_maps.append({
            "xa": np.ascontiguousarray(
                x[b].T.reshape(KCH, 128, L)).astype(BF16),
            "wq": pack_k(w_qkv[r].T),
            "wk": pack_k(w_qkv[C:][r].T),
            "wv": pack_k(w_qkv[2 * C:][r].T),
            "wp": np.ascontiguousarray(
                w_proj[:, r].T.reshape(2, 128, C).transpose(1, 0, 2)).astype(BF16),
            "bb": np.ascontiguousarray(bqk, np.float32),
            "cs": cs,
        })
    return in_maps


def kernel(x, w_qkv, b_qkv, w_proj, cos, sin, mask=None, trace=False):
    nc = _get_program()
    in_maps = make_core_inputs(x, w_qkv, b_qkv, w_proj, cos, sin)
    res = bass_utils.run_bass_kernel_spmd(
        nc, in_maps, core_ids=list(range(N_CORES)), trace=trace)
    # v-bias enters the output linearly: + b_v @ w_proj.T on every row
    bias_row = (np.asarray(b_qkv, np.float32)[2 * C:]
                @ np.asarray(w_proj, np.float32).T)
    out = np.zeros((B, L_FULL, C), np.float32)
    for c in range(N_CORES):
        out[c // 4] += res.results[c]["o"].astype(np.float32).reshape(L_FULL, C)
    out += bias_row[None, None, :]
    if trace:
        kernel.last_results = res
    return out



# revision 29
# speedup vs baseline: 1.1111x; 1.0106x over previous
"""Trainium2 Bass kernel for nn_BoxAwareAttention: full attention block
(QKV proj + bias, RoPE, scaled-dot-product attention with softmax, out proj).

Sharding over 8 NeuronCores: data-parallel over batch (2) x tensor-parallel
over heads (16 -> 4 per core).  Core c handles batch c//4, heads 4*(c%4)..+4.
Each core computes its partial projection output (contraction over its 256
channels); host sums the 4 partials per batch and adds the (linear) v-bias
contribution b_v @ w_proj.T as a constant row.

v10 schedule.  Hardware facts driving it: (1) a HAM power governor clips
the PE to half clock when it sustains high duty over ~20-40us stretches
(energy-bucket-like, with 3.4us granularity and long hysteresis) — the
run is in effect PE-energy-bound, so every PE-busy ns counts ~1.3x;
(2) DVE ops complete-serialize (~1.1us per full-width op), so a q/k
tile's RoPE chain (4 partition-shifted rotate-half reads from PSUM + cos
multiply, DVE-only: GpSimd cannot touch PSUM) is ~5.3us of serial DVE;
(3) ACT is the only exp engine: 128 exps x 1.11us = 142.5us is the
attention-phase floor (steady state is ACT-paced at ~1.29us/k-tile);
(4) the PE executes its stream IN ORDER: any pumped matmul whose PSUM
slot is still being read head-of-line-blocks the whole window, so the
feed queue spaces feeders with nop slots sized to the ps_proj drain.

  - Prologue holds only what attention(qb0, h0..1) strictly needs before
    its kt10: kT pair-0 and qT(block0, pair0) rope chains plus all of V;
    xa/rope tables are DMA'd in halves so ~3.7MB gates the first K
    projection, and qT(mt1,b0)'s fed matmuls are pumped into the DMA
    shadow.  Everything else — kT pair-1, block-1 q tiles, qb0's
    out-proj (2 tiles/window over (1,0)-(1,3)) — chains one tile at a
    time through the ps_proj slot at 2 pops per k-tile step (~13-16 kt
    per tile: 8kt of matmuls + rope reads + nop spacing).
  - PV trails QK/exp by 10 k-tiles; each window's last 10 PV matmuls
    are flushed 2-per-kt at kt1-5 of the NEXT window and its norm at
    kt6 (fins are held while a carry is pending so the norm's DVE ops
    aren't queued behind a rope chain — pos recycling gates kt10's
    first PV).  The last window trails only 4 so its PVs finish mostly
    in-window.
  - v natural [L, d] (bias applied on host) with an appended ones column:
    the softmax denominator falls out of the P@V matmul as row 64; plain
    DVE copy stages it to base partition 0 (custom-DVE ops mishandle
    nonzero AP bases), fast reciprocal, partition_broadcast on GpSimd.
  - q-block 1's out-proj ping-pongs PSUM between the proj pool and the
    then-idle score pool, interleaved with the final norm halves (qt8-11
    need only sp0 of oT), evacuations alternating ACT/DVE.  Output DMA'd
    as bf16.
"""

import os
import sys

for _p in ("/opt/trn_rl_repo", "/root/.axon_site/_ro/trn_rl_repo"):
    if os.path.isdir(_p) and _p not in sys.path:
        sys.path.insert(0, _p)

import numpy as np
import ml_dtypes

import concourse.bass as bass
import concourse.mybir as mybir
import concourse.tile as tile
from concourse import bacc
from concourse import bass_utils

BF16 = ml_dtypes.bfloat16
N_CORES = 8
B, L_FULL, C, H, D = 2, 2048, 1024, 16, 64
H_LOC = 4              # heads per core
M_LOC = H_LOC * D      # 256 output channels per core
KCH = 8                # qkv contraction chunks (1024 = 8*128)
TRAIL = 10             # PV lag behind QK/exp, in k-tiles
F32 = mybir.dt.float32
BF = mybir.dt.bfloat16
ADD = mybir.AluOpType.add
MULT = mybir.AluOpType.mult


def build_program(L=L_FULL, num_devices=N_CORES):
    """Build the per-core Bass program (SPMD: same program, per-core data)."""
    NB = min(1024, L)      # q/L block width (PSUM tile free size)
    nNB = L // NB          # number of blocks
    nKT = L // 128         # attention k-tiles / v L-tiles
    nMT = 2                # q/k M-tiles (2 heads of 64 each)
    nQT = L // 128         # proj q-tiles
    PW = min(512, NB)      # PV/norm sub-block width
    SPL = [bass.ds(s, PW) for s in range(0, NB, PW)]

    nc = bacc.Bacc("TRN2", target_bir_lowering=False, debug=False,
                   num_devices=num_devices)

    xa_d = nc.dram_tensor("xa", [KCH, 128, L], BF, kind="ExternalInput").ap()
    wq_d = nc.dram_tensor("wq", [128, KCH, M_LOC], BF, kind="ExternalInput").ap()
    wk_d = nc.dram_tensor("wk", [128, KCH, M_LOC], BF, kind="ExternalInput").ap()
    wv_d = nc.dram_tensor("wv", [128, KCH, M_LOC], BF, kind="ExternalInput").ap()
    wp_d = nc.dram_tensor("wp", [128, 2, C], BF, kind="ExternalInput").ap()
    bb_d = nc.dram_tensor("bb", [128, 4], F32, kind="ExternalInput").ap()
    cs_d = nc.dram_tensor("cs", [128, 2, L], BF, kind="ExternalInput").ap()
    o_d = nc.dram_tensor("o", [nQT, 128, C], BF, kind="ExternalOutput").ap()

    with tile.TileContext(nc) as tc:
        with (
            tc.tile_pool(name="const", bufs=1) as cpool,
            tc.tile_pool(name="rope", bufs=3) as rpool,
            tc.tile_pool(name="pt", bufs=12) as ptpool,
            tc.tile_pool(name="norm", bufs=3) as npool,
            tc.tile_pool(name="outs", bufs=3) as opool,
            tc.tile_pool(name="ps_big", bufs=2, space="PSUM") as ps_big,
            tc.tile_pool(name="ps_proj", bufs=1, space="PSUM") as ps_proj,
            tc.tile_pool(name="ps_o", bufs=2, space="PSUM") as ps_o,
        ):
            xa = [cpool.tile([128, L], BF, tag=f"xa{c}", name=f"xa{c}")
                  for c in range(KCH)]
            wq = cpool.tile([128, KCH, M_LOC], BF, tag="wq")
            wk = cpool.tile([128, KCH, M_LOC], BF, tag="wk")
            wv = cpool.tile([128, KCH, M_LOC], BF, tag="wv")
            wp = cpool.tile([128, 2, C], BF, tag="wp")
            bb = cpool.tile([128, 4], F32, tag="bb")
            cs = cpool.tile([128, 2, L], BF, tag="cs")
            qT = [cpool.tile([128, L], BF, tag=f"qT{m}", name=f"qT{m}") for m in range(nMT)]
            kT = [cpool.tile([128, L], BF, tag=f"kT{m}", name=f"kT{m}") for m in range(nMT)]
            oT = [cpool.tile([128, L], BF, tag=f"oT{m}", name=f"oT{m}") for m in range(nMT)]
            v_aug = cpool.tile([128, nKT, H_LOC, 65], BF, tag="vaug")

            # input DMAs ordered by first use; xa split by column half so
            # the first K projection isn't gated on the full activation load.
            # cs split by L-half too: only the lb0 half gates the first ropes.
            h0c = bass.ds(0, NB)
            h1c = bass.ds(NB, NB)
            nc.sync.dma_start(wk[:], wk_d[:])
            nc.sync.dma_start(xa[0][:, h0c], xa_d[0][:, h0c])
            nc.sync.dma_start(cs[:, :, h0c], cs_d[:, :, h0c])
            nc.sync.dma_start(bb[:], bb_d[:])
            nc.sync.dma_start(wq[:], wq_d[:])
            for c in range(1, KCH):
                nc.sync.dma_start(xa[c][:, h0c], xa_d[c][:, h0c])
            nc.sync.dma_start(wv[:], wv_d[:])
            for c in range(KCH):
                nc.sync.dma_start(xa[c][:, h1c], xa_d[c][:, h1c])
            nc.sync.dma_start(cs[:, :, h1c], cs_d[:, :, h1c])
            nc.sync.dma_start(wp[:], wp_d[:])
            nc.vector.memset(v_aug[:, :, :, 64:65], 1.0)

            cos_s = cs[:, 0, :]
            sinx_s = cs[:, 1, :]

            # ---- q/k projection tile: matmuls + bias + RoPE ----
            qk_idx = [0]
            def qk_mms(pq, w_s, mt, ls):
                for cc in range(KCH):
                    for sp in SPL:
                        nc.tensor.matmul(
                            pq[:, sp], w_s[:, cc, bass.ts(mt, 128)],
                            xa[cc][:, ls][:, sp],
                            start=(cc == 0), stop=(cc == KCH - 1))

            def qk_fin(pq, w_s, dstT, mt, ls, tag):
                # bias in-place on PSUM; rotate-half = partition-shifted
                # PSUM reads (DVE only)
                ti = 0 if w_s is wq else 1
                nc.scalar.activation(
                    pq[:], pq[:],
                    mybir.ActivationFunctionType.Identity,
                    bias=bb[:, ti * 2 + mt:ti * 2 + mt + 1])
                rot = rpool.tile([128, NB], BF, tag="rot", name=f"rot{tag}")
                for do, so in ((0, 32), (32, 0), (64, 96), (96, 64)):
                    nc.vector.tensor_tensor(
                        rot[do:do + 32, :], pq[so:so + 32, :],
                        sinx_s[do:do + 32, ls], MULT)
                tcos = rpool.tile([128, NB], BF, tag="tcos", name=f"tcos{tag}")
                nc.vector.tensor_tensor(tcos[:], pq[:], cos_s[:, ls], MULT)
                nc.gpsimd.tensor_tensor(dstT[mt][:, ls], tcos[:], rot[:], ADD)

            def qk_tile(w_s, dstT, mt, lb, pool=None):
                # prologue tiles rotate the score pool by default, keeping
                # the proj slot free for the first feeder matmuls
                ls = bass.ds(lb * NB, NB)
                pq = (pool or ps_big).tile(
                    [128, NB], F32, tag="pp" if pool is ps_proj else "big")
                qk_idx[0] += 1
                qk_mms(pq, w_s, mt, ls)
                qk_fin(pq, w_s, dstT, mt, ls, f"p{mt}{lb}")

            # ---- one V tile: natural [L, d], no bias, ACT evac, ps_o ----
            def v_tile(lt):
                pv = ps_o.tile([128, M_LOC], F32, tag="po", name=f"pv{lt}")
                for cc in range(KCH):
                    nc.tensor.matmul(
                        pv[:], xa[cc][:, bass.ts(lt, 128)], wv[:, cc, :],
                        start=(cc == 0), stop=(cc == KCH - 1))
                nc.scalar.copy(
                    v_aug[:, lt, :, 0:64],
                    pv[:].rearrange("p (h d) -> p h d", h=H_LOC))

            # ---- out-proj pieces ----
            def oproj_mms(pp, qt):
                for cc in range(2):
                    for nn in range(2):
                        nc.tensor.matmul(
                            pp[:, bass.ts(nn, C // 2)],
                            oT[cc][:, bass.ts(qt, 128)],
                            wp[:, cc, bass.ts(nn, C // 2)],
                            start=(cc == 0), stop=(cc == 1))

            def oproj_evac(pp, qt, evac):
                ost = opool.tile([128, C], BF, tag="ost", name=f"ost{qt}")
                if evac == "act":
                    nc.scalar.copy(ost[:], pp[:])
                else:
                    nc.vector.tensor_copy(ost[:], pp[:])
                nc.sync.dma_start(o_d[qt], ost[:])

            # ---- feeder: deferred work, 1-2 matmuls per k-tile step; DVE
            # finishers (rope / evac) held until the window's norm is issued
            feed_q = []

            def feed_qk(w_s, dstT, mt, lb, tail_nops=12):
                ls = bass.ds(lb * NB, NB)
                st8 = {}
                def alloc():
                    if "pq" not in st8:
                        st8["pq"] = ps_proj.tile(
                            [128, NB], F32, tag="pp", name=f"fq{mt}{lb}")
                    return st8["pq"]
                for cc in range(KCH):
                    for sp in SPL:
                        def mm(cc=cc, sp=sp):
                            nc.tensor.matmul(
                                alloc()[:, sp], w_s[:, cc, bass.ts(mt, 128)],
                                xa[cc][:, ls][:, sp],
                                start=(cc == 0), stop=(cc == KCH - 1))
                        feed_q.append({"f": mm, "fin": False})
                def fin():
                    qk_fin(st8["pq"], w_s, dstT, mt, ls, f"f{mt}{lb}")
                feed_q.append({"f": fin, "fin": True})
                # the rope chain reads the ps_proj tile for ~5 DVE ops
                # (~5-6 kt); hold the next feeder off that long
                feed_nops(tail_nops)

            def feed_nops(n):
                # spacing entries: burn pump slots so the next feeder's
                # matmuls aren't popped before the ps_proj slot is free
                # (a blocked matmul stalls the in-order PE stream)
                for _ in range(n):
                    feed_q.append({"f": lambda: None, "fin": False})

            def feed_oproj(qt):
                st8 = {}
                def alloc():
                    if "pp" not in st8:
                        st8["pp"] = ps_proj.tile(
                            [128, C], F32, tag="pp", name=f"fo{qt}")
                    return st8["pp"]
                for cc in range(2):
                    for nn in range(2):
                        def mm(cc=cc, nn=nn):
                            nc.tensor.matmul(
                                alloc()[:, bass.ts(nn, C // 2)],
                                oT[cc][:, bass.ts(qt, 128)],
                                wp[:, cc, bass.ts(nn, C // 2)],
                                start=(cc == 0), stop=(cc == 1))
                        feed_q.append({"f": mm, "fin": False})
                def fin():
                    oproj_evac(st8["pp"], qt, "dve")
                feed_q.append({"f": fin, "fin": True})
                feed_nops(2)

            def pump(kt, n, block_fins=False):
                for _ in range(n):
                    if not feed_q:
                        return
                    if feed_q[0]["fin"] and block_fins:
                        return
                    feed_q.pop(0)["f"]()

            # ---- prologue: only what window (0,0) needs before its kt10:
            # kT(pair0), qT(block0, pair0), and all of V.  Everything else
            # flows through the feed queue: qT(mt1,b0) / kT(pair1) / the
            # block-1 q tiles chain one-at-a-time through the single
            # ps_proj slot (~13 kt each: 8 kt of matmuls + rope reads),
            # so the chain starts in the DMA-shadowed prologue (PE is
            # half idle there) via explicit pump calls, sized so only
            # q10's entries drain before the window loop.
            feed_qk(wq, qT, 1, 0, tail_nops=20)
            feed_qk(wk, kT, 1, 0, tail_nops=12)
            feed_qk(wk, kT, 1, 1, tail_nops=16)
            # k00, q00 and q10's fed matmuls all pace with the per-chunk
            # xa arrivals — they fill the DMA shadow; v and k01 follow.
            # The window loop starts right after v15, gated by q00's rope.
            qk_tile(wk, kT, 0, 0)
            qk_tile(wq, qT, 0, 0)
            pump(-1, 16)
            for lt in range(12):
                v_tile(lt)
            qk_tile(wk, kT, 0, 1)
            pump(-1, 8)
            for lt in range(12, nKT):
                v_tile(lt)
                pump(-1, 2)

            # remaining per-window feeder registration: q01 during (0,1)
            # (consumed from (1,0)), q11 during (0,2) (consumed (1,2)),
            # qb0's out-proj spread 2 tiles/window over (1,0)-(1,3).
            feeders = {
                (0, 1): [(feed_qk, (wq, qT, 0, 1))],
                (0, 2): [(feed_qk, (wq, qT, 1, 1))],
                (1, 0): [(feed_oproj, (qt,)) for qt in range(2)],
                (1, 1): [(feed_oproj, (qt,)) for qt in range(2, 4)],
                (1, 2): [(feed_oproj, (qt,)) for qt in range(4, 6)],
                (1, 3): [(feed_oproj, (qt,)) for qt in range(6, 8)],
            }

            def norm_head_sp(pos_c, mt_c, pr_c, qs_c, si):
                sp = SPL[si]
                po = pos_c[si]
                # softmax denominator: row 64 (ones column of v_aug);
                # staged to base partition 0 by a plain DVE copy
                dn = npool.tile([1, PW], F32, tag="dn")
                nc.vector.tensor_copy(dn[:], po[64:65, :])
                rc = npool.tile([1, PW], F32, tag="rc")
                nc.vector.reciprocal_approx_fast(rc[:], dn[:])
                rb = npool.tile([64, PW], F32, tag="rb")
                nc.gpsimd.partition_broadcast(rb[:], rc[:], channels=64)
                nc.vector.tensor_tensor(
                    oT[mt_c][pr_c, qs_c][:, sp], po[0:64, :], rb[:], MULT)

            def norm_head(pos_c, mt_c, pr_c, qs_c):
                for si in range(len(SPL)):
                    norm_head_sp(pos_c, mt_c, pr_c, qs_c, si)

            def pv_issue(pos_w, h_w, pts_w, ktc, last):
                for si, sp in enumerate(SPL):
                    nc.tensor.matmul(
                        pos_w[si][:], v_aug[:, ktc, h_w, :],
                        pts_w[ktc][:, sp],
                        start=(ktc == 0), stop=(ktc == nKT - 1 and last))

            # ---- attention: exps never pause; PV trails by TRAIL k-tiles
            # and each window's trailing PVs + norm flush early in the next
            carry = None
            for qb in range(nNB):
                qs = bass.ds(qb * NB, NB)
                for h in range(H_LOC):
                    mt, hh = divmod(h, 2)
                    pr = slice(64 * hh, 64 * hh + 64)
                    for fn, args in feeders.get((qb, h), []):
                        fn(*args)
                    pos = [ps_o.tile([65, PW], F32, tag="po",
                                     name=f"po{qb}{h}{si}")
                           for si in range(len(SPL))]
                    pts = {}
                    for kt in range(nKT):
                        st = ps_big.tile([128, NB], F32, tag="big")
                        for sp in SPL:
                            nc.tensor.matmul(
                                st[:, sp], kT[mt][pr, bass.ts(kt, 128)],
                                qT[mt][pr, qs][:, sp],
                                start=True, stop=True)
                        pt = ptpool.tile([128, NB], BF, tag="pt")
                        nc.scalar.activation(
                            pt[:], st[:], mybir.ActivationFunctionType.Exp,
                            scale=float(D) ** -0.5)
                        pts[kt] = pt
                        if carry is not None and 1 <= kt <= 6:
                            c_pos, c_h, c_pts, c_mt, c_pr, c_qs = carry
                            if kt <= 5:
                                for ktc in (nKT - TRAIL + 2 * kt - 2,
                                            nKT - TRAIL + 2 * kt - 1):
                                    pv_issue(c_pos, c_h, c_pts, ktc, True)
                            else:
                                norm_head(c_pos, c_mt, c_pr, c_qs)
                                carry = None
                        # last window trails less so its PVs mostly finish
                        # inside the window (shorter serial tail)
                        trail_w = 4 if (qb, h) == (nNB - 1, H_LOC - 1) else TRAIL
                        if kt >= trail_w:
                            pv_issue(pos, h, pts, kt - trail_w, False)
                        # (1,0) feeders read qb0's oT, whose last head's
                        # norm is carry-flushed at kt6 — no pumping before
                        # that (a blocked matmul stalls the in-order PE).
                        # While a carry is pending, hold rope/evac fins so
                        # the carried norm's DVE ops aren't queued behind a
                        # 5-op rope chain (pos recycling gates next PV).
                        if (qb, h) != (1, 0) or kt >= 7:
                            pump(kt, 2, block_fins=(carry is not None))
                    carry = (pos, h, pts, mt, pr, qs)
            # final window: flush trailing PVs si-major so norm(sp0) issues
            # while si1's PVs still stream on the PE; interleave the two
            # norm halves with the out-proj tiles they gate (qt8-11 read
            # only sp0 columns of oT, qt12-15 only sp1), and spread the
            # evacuations over ACT and DVE so the tail isn't one-engine
            # paced.
            c_pos, c_h, c_pts, c_mt, c_pr, c_qs = carry
            for si, sp in enumerate(SPL):
                for ktc in range(nKT - 4, nKT):
                    nc.tensor.matmul(
                        c_pos[si][:], v_aug[:, ktc, c_h, :],
                        c_pts[ktc][:, sp],
                        start=False, stop=(ktc == nKT - 1))
                if si == 0:
                    norm_head_sp(c_pos, c_mt, c_pr, c_qs, 0)
            while feed_q:
                feed_q.pop(0)["f"]()

            # tail: q-block 1's out-proj, ping-pong proj/score PSUM pools,
            # evacuations alternating ACT/DVE so neither engine paces it
            for j in range(NB // 128):
                qt = NB // 128 + j
                pp = (ps_proj if j % 2 == 0 else ps_big).tile(
                    [128, C], F32, tag="pp" if j % 2 == 0 else "big",
                    name=f"tp{qt}")
                oproj_mms(pp, qt)
                oproj_evac(pp, qt, "act" if j % 2 == 0 else "dve")
                if j == 3:
                    norm_head_sp(c_pos, c_mt, c_pr, c_qs, 1)

    nc.compile()
    return nc


_CACHE = {}


def _get_program(L=L_FULL):
    if L not in _CACHE:
        _CACHE[L] = build_program(L)
    return _CACHE[L]


def make_core_inputs(x, w_qkv, b_qkv, w_proj, cos, sin, L=L_FULL):
    """Host-side shard/transpose/pad/cast. Returns in_maps for the 8 cores."""
    x = np.asarray(x, np.float32)
    w_qkv = np.asarray(w_qkv, np.float32)
    b_qkv = np.asarray(b_qkv, np.float32)
    w_proj = np.asarray(w_proj, np.float32)
    cos = np.asarray(cos, np.float32)
    sin = np.asarray(sin, np.float32)

    # replicated rope tables: [128, L] (2 heads stacked), sign folded into sin
    cT = cos.T.astype(np.float32)                      # [64, L]
    sT = sin.T.astype(np.float32)
    cosT = np.concatenate([cT, cT], 0)                 # [128, L]
    sx = np.concatenate([-sT[0:32], sT[32:64]], 0)
    sinX = np.concatenate([sx, sx], 0)
    cs = np.ascontiguousarray(
        np.stack([cosT, sinX], axis=1)).astype(BF16)   # [128, 2, L]

    def pack_k(mat_t):
        # mat_t: [C, M] (already transposed) -> [128, KCH, M] bf16
        Cdim, M = mat_t.shape
        assert Cdim == KCH * 128
        return np.ascontiguousarray(
            mat_t.reshape(KCH, 128, M).transpose(1, 0, 2)).astype(BF16)

    in_maps = []
    for c in range(N_CORES):
        b, hg = divmod(c, 4)
        h0 = H_LOC * hg
        r = slice(h0 * D, (h0 + H_LOC) * D)            # head-channel rows
        # per-partition q/k biases for the [d, L] layout (cols: q-mt0,
        # q-mt1, k-mt0, k-mt1); v bias is applied on the host
        bq, bk = b_qkv[r], b_qkv[C:][r]
        bqk = np.stack([bq[:128], bq[128:], bk[:128], bk[128:]], 1)
        in_maps.append({
            "xa": np.ascontiguousarray(
                x[b].T.reshape(KCH, 128, L)).astype(BF16),
            "wq": pack_k(w_qkv[r].T),
            "wk": pack_k(w_qkv[C:][r].T),
            "wv": pack_k(w_qkv[2 * C:][r].T),
            "wp": np.ascontiguousarray(
                w_proj[:, r].T.reshape(2, 128, C).transpose(1, 0, 2)).astype(BF16),
            "bb": np.ascontiguousarray(bqk, np.float32),
            "cs": cs,
        })
    return in_maps


def kernel(x, w_qkv, b_qkv, w_proj, cos, sin, mask=None, trace=False):
    nc = _get_program()
    in_maps = make_core_inputs(x, w_qkv, b_qkv, w_proj, cos, sin)
    res = bass_utils.run_bass_kernel_spmd(
        nc, in_maps, core_ids=list(range(N_CORES)), trace=trace)
    # v-bias enters the output linearly: + b_v @ w_proj.T on every row
    bias_row = (np.asarray(b_qkv, np.float32)[2 * C:]
                @ np.asarray(w_proj, np.float32).T)
    out = np.zeros((B, L_FULL, C), np.float32)
    for c in range(N_CORES):
        out[c // 4] += res.results[c]["o"].astype(np.float32).reshape(L_FULL, C)
    out += bias_row[None, None, :]
    if trace:
        kernel.last_results = res
    return out



# revision 35
# speedup vs baseline: 1.2831x; 1.1549x over previous
"""Trainium2 Bass kernel for nn_BoxAwareAttention: full attention block
(QKV proj + bias, RoPE, scaled-dot-product attention with softmax, out proj).

Sharding over 8 NeuronCores: data-parallel over batch (2) x tensor-parallel
over heads (16 -> 4 per core).  Core c handles batch c//4, heads 4*(c%4)..+4.
Each core computes its partial projection output (contraction over its 256
channels); host sums the 4 partials per batch and adds the (linear) v-bias
contribution b_v @ w_proj.T as a constant row.

v10 schedule.  Hardware facts driving it: (1) a HAM power governor clips
the PE to half clock when it sustains high duty over ~20-40us stretches
(energy-bucket-like, with 3.4us granularity and long hysteresis) — the
run is in effect PE-energy-bound, so every PE-busy ns counts ~1.3x;
(2) DVE ops complete-serialize (~1.1us per full-width op), so a q/k
tile's RoPE chain (4 partition-shifted rotate-half reads from PSUM + cos
multiply, DVE-only: GpSimd cannot touch PSUM) is ~5.3us of serial DVE;
(3) ACT is the only exp engine: 128 exps x 1.11us = 142.5us is the
attention-phase floor (steady state is ACT-paced at ~1.29us/k-tile);
(4) the PE executes its stream IN ORDER: any pumped matmul whose PSUM
slot is still being read head-of-line-blocks the whole window, so the
feed queue spaces feeders with nop slots sized to the ps_proj drain.

  - Prologue holds only what attention(qb0, h0..1) strictly needs before
    its kt10: kT pair-0 and qT(block0, pair0) rope chains plus all of V;
    xa/rope tables are DMA'd in halves so ~3.7MB gates the first K
    projection, and qT(mt1,b0)'s fed matmuls are pumped into the DMA
    shadow.  Everything else — kT pair-1, block-1 q tiles, qb0's
    out-proj (2 tiles/window over (1,0)-(1,3)) — chains one tile at a
    time through the ps_proj slot at 2 pops per k-tile step (~13-16 kt
    per tile: 8kt of matmuls + rope reads + nop spacing).
  - PV trails QK/exp by 10 k-tiles; each window's last 10 PV matmuls
    are flushed 2-per-kt at kt1-5 of the NEXT window and its norm at
    kt6 (fins are held while a carry is pending so the norm's DVE ops
    aren't queued behind a rope chain — pos recycling gates kt10's
    first PV).  The last window trails only 4 so its PVs finish mostly
    in-window.
  - v natural [L, d] (bias applied on host) with 64 PREPENDED ones
    columns: the P@V matmul replicates the softmax denominator across
    out partitions 0-63 at no PE cost (matmul cost = moving columns),
    so the norm is 3 all-DVE ops (copy to base-0 SBUF for the custom
    reciprocal, reciprocal, multiply) with no GpSimd broadcast hop —
    the old 4-op 2-engine chain was the qb-boundary critical path.
  - q-block 1's out-proj ping-pongs PSUM between the proj pool and the
    then-idle score pool, interleaved with the final norm halves (qt8-11
    need only sp0 of oT), evacuations alternating ACT/DVE.  Output DMA'd
    as bf16.
"""

import os
import sys

for _p in ("/opt/trn_rl_repo", "/root/.axon_site/_ro/trn_rl_repo"):
    if os.path.isdir(_p) and _p not in sys.path:
        sys.path.insert(0, _p)

import numpy as np
import ml_dtypes

import concourse.bass as bass
import concourse.mybir as mybir
import concourse.tile as tile
from concourse import bacc
from concourse import bass_utils

BF16 = ml_dtypes.bfloat16
N_CORES = 8
B, L_FULL, C, H, D = 2, 2048, 1024, 16, 64
H_LOC = 4              # heads per core
M_LOC = H_LOC * D      # 256 output channels per core
KCH = 8                # qkv contraction chunks (1024 = 8*128)
TRAIL = 10             # PV lag behind QK/exp, in k-tiles
F32 = mybir.dt.float32
BF = mybir.dt.bfloat16
ADD = mybir.AluOpType.add
MULT = mybir.AluOpType.mult


def build_program(L=L_FULL, num_devices=N_CORES):
    """Build the per-core Bass program (SPMD: same program, per-core data)."""
    NB = min(1024, L)      # q/L block width (PSUM tile free size)
    nNB = L // NB          # number of blocks
    nKT = L // 128         # attention k-tiles / v L-tiles
    nMT = 2                # q/k M-tiles (2 heads of 64 each)
    nQT = L // 128         # proj q-tiles
    PW = min(512, NB)      # PV/norm sub-block width
    SPL = [bass.ds(s, PW) for s in range(0, NB, PW)]

    nc = bacc.Bacc("TRN2", target_bir_lowering=False, debug=False,
                   num_devices=num_devices)

    xa_d = nc.dram_tensor("xa", [KCH, 128, L], BF, kind="ExternalInput").ap()
    wq_d = nc.dram_tensor("wq", [128, KCH, M_LOC], BF, kind="ExternalInput").ap()
    wk_d = nc.dram_tensor("wk", [128, KCH, M_LOC], BF, kind="ExternalInput").ap()
    wv_d = nc.dram_tensor("wv", [128, KCH, M_LOC], BF, kind="ExternalInput").ap()
    wp_d = nc.dram_tensor("wp", [128, 2, C], BF, kind="ExternalInput").ap()
    bb_d = nc.dram_tensor("bb", [128, 4], F32, kind="ExternalInput").ap()
    cs_d = nc.dram_tensor("cs", [128, 2, L], BF, kind="ExternalInput").ap()
    o_d = nc.dram_tensor("o", [nQT, 128, C], BF, kind="ExternalOutput").ap()

    with tile.TileContext(nc) as tc:
        with (
            tc.tile_pool(name="const", bufs=1) as cpool,
            tc.tile_pool(name="rope", bufs=3) as rpool,
            tc.tile_pool(name="pt", bufs=12) as ptpool,
            tc.tile_pool(name="norm", bufs=3) as npool,
            tc.tile_pool(name="outs", bufs=3) as opool,
            tc.tile_pool(name="ps_big", bufs=2, space="PSUM") as ps_big,
            tc.tile_pool(name="ps_proj", bufs=1, space="PSUM") as ps_proj,
            tc.tile_pool(name="ps_o", bufs=2, space="PSUM") as ps_o,
        ):
            xa = [cpool.tile([128, L], BF, tag=f"xa{c}", name=f"xa{c}")
                  for c in range(KCH)]
            wq = cpool.tile([128, KCH, M_LOC], BF, tag="wq")
            wk = cpool.tile([128, KCH, M_LOC], BF, tag="wk")
            wv = cpool.tile([128, KCH, M_LOC], BF, tag="wv")
            wp = cpool.tile([128, 2, C], BF, tag="wp")
            bb = cpool.tile([128, 4], F32, tag="bb")
            cs = cpool.tile([128, 2, L], BF, tag="cs")
            qT = [cpool.tile([128, L], BF, tag=f"qT{m}", name=f"qT{m}") for m in range(nMT)]
            kT = [cpool.tile([128, L], BF, tag=f"kT{m}", name=f"kT{m}") for m in range(nMT)]
            oT = [cpool.tile([128, L], BF, tag=f"oT{m}", name=f"oT{m}") for m in range(nMT)]
            # cols 0-63 all-ones, 64-127 the V values: the P@V matmul then
            # yields the softmax denominator REPLICATED on out partitions
            # 0-63 (base 0, as reciprocal_approx_fast requires) at no PE
            # cost (matmul cost = moving columns), killing the GpSimd
            # partition_broadcast hop in the norm chain.
            v_aug = cpool.tile([128, nKT, H_LOC, 128], BF, tag="vaug")

            # input DMAs ordered by first use; xa split by column half so
            # the first K projection isn't gated on the full activation load.
            # cs split by L-half too: only the lb0 half gates the first ropes.
            h0c = bass.ds(0, NB)
            h1c = bass.ds(NB, NB)
            nc.sync.dma_start(wk[:], wk_d[:])
            nc.sync.dma_start(xa[0][:, h0c], xa_d[0][:, h0c])
            nc.sync.dma_start(cs[:, :, h0c], cs_d[:, :, h0c])
            nc.sync.dma_start(bb[:], bb_d[:])
            nc.sync.dma_start(wq[:], wq_d[:])
            for c in range(1, KCH):
                nc.sync.dma_start(xa[c][:, h0c], xa_d[c][:, h0c])
            nc.sync.dma_start(wv[:], wv_d[:])
            for c in range(KCH):
                nc.sync.dma_start(xa[c][:, h1c], xa_d[c][:, h1c])
            nc.sync.dma_start(cs[:, :, h1c], cs_d[:, :, h1c])
            nc.sync.dma_start(wp[:], wp_d[:])
            nc.vector.memset(v_aug[:, :, :, 0:64], 1.0)

            cos_s = cs[:, 0, :]
            sinx_s = cs[:, 1, :]

            # ---- q/k projection tile: matmuls + bias + RoPE ----
            qk_idx = [0]
            def qk_mms(pq, w_s, mt, ls):
                for cc in range(KCH):
                    for sp in SPL:
                        nc.tensor.matmul(
                            pq[:, sp], w_s[:, cc, bass.ts(mt, 128)],
                            xa[cc][:, ls][:, sp],
                            start=(cc == 0), stop=(cc == KCH - 1))

            def qk_fin(pq, w_s, dstT, mt, ls, tag):
                # bias in-place on PSUM; rotate-half = partition-shifted
                # PSUM reads (DVE only)
                ti = 0 if w_s is wq else 1
                nc.scalar.activation(
                    pq[:], pq[:],
                    mybir.ActivationFunctionType.Identity,
                    bias=bb[:, ti * 2 + mt:ti * 2 + mt + 1])
                rot = rpool.tile([128, NB], BF, tag="rot", name=f"rot{tag}")
                for do, so in ((0, 32), (32, 0), (64, 96), (96, 64)):
                    nc.vector.tensor_tensor(
                        rot[do:do + 32, :], pq[so:so + 32, :],
                        sinx_s[do:do + 32, ls], MULT)
                tcos = rpool.tile([128, NB], BF, tag="tcos", name=f"tcos{tag}")
                nc.vector.tensor_tensor(tcos[:], pq[:], cos_s[:, ls], MULT)
                nc.gpsimd.tensor_tensor(dstT[mt][:, ls], tcos[:], rot[:], ADD)

            def qk_tile(w_s, dstT, mt, lb, pool=None):
                # prologue tiles rotate the score pool by default, keeping
                # the proj slot free for the first feeder matmuls
                ls = bass.ds(lb * NB, NB)
                pq = (pool or ps_big).tile(
                    [128, NB], F32, tag="pp" if pool is ps_proj else "big")
                qk_idx[0] += 1
                qk_mms(pq, w_s, mt, ls)
                qk_fin(pq, w_s, dstT, mt, ls, f"p{mt}{lb}")

            # ---- one V tile: natural [L, d], no bias, ACT evac, ps_o ----
            def v_tile(lt):
                pv = ps_o.tile([128, M_LOC], F32, tag="po", name=f"pv{lt}")
                for cc in range(KCH):
                    nc.tensor.matmul(
                        pv[:], xa[cc][:, bass.ts(lt, 128)], wv[:, cc, :],
                        start=(cc == 0), stop=(cc == KCH - 1))
                nc.scalar.copy(
                    v_aug[:, lt, :, 64:128],
                    pv[:].rearrange("p (h d) -> p h d", h=H_LOC))

            # ---- out-proj pieces ----
            def oproj_mms(pp, qt):
                for cc in range(2):
                    for nn in range(2):
                        nc.tensor.matmul(
                            pp[:, bass.ts(nn, C // 2)],
                            oT[cc][:, bass.ts(qt, 128)],
                            wp[:, cc, bass.ts(nn, C // 2)],
                            start=(cc == 0), stop=(cc == 1))

            def oproj_evac(pp, qt, evac):
                ost = opool.tile([128, C], BF, tag="ost", name=f"ost{qt}")
                if evac == "act":
                    nc.scalar.copy(ost[:], pp[:])
                else:
                    nc.vector.tensor_copy(ost[:], pp[:])
                nc.sync.dma_start(o_d[qt], ost[:])

            # ---- feeder: deferred work, 1-2 matmuls per k-tile step; DVE
            # finishers (rope / evac) held until the window's norm is issued
            feed_q = []

            def feed_qk(w_s, dstT, mt, lb, tail_nops=12):
                ls = bass.ds(lb * NB, NB)
                st8 = {}
                def alloc():
                    if "pq" not in st8:
                        st8["pq"] = ps_proj.tile(
                            [128, NB], F32, tag="pp", name=f"fq{mt}{lb}")
                    return st8["pq"]
                for cc in range(KCH):
                    for sp in SPL:
                        def mm(cc=cc, sp=sp):
                            nc.tensor.matmul(
                                alloc()[:, sp], w_s[:, cc, bass.ts(mt, 128)],
                                xa[cc][:, ls][:, sp],
                                start=(cc == 0), stop=(cc == KCH - 1))
                        feed_q.append({"f": mm, "fin": False})
                def fin():
                    qk_fin(st8["pq"], w_s, dstT, mt, ls, f"f{mt}{lb}")
                feed_q.append({"f": fin, "fin": True})
                # the rope chain reads the ps_proj tile for ~5 DVE ops
                # (~5-6 kt); hold the next feeder off that long
                feed_nops(tail_nops)

            def feed_nops(n):
                # spacing entries: burn pump slots so the next feeder's
                # matmuls aren't popped before the ps_proj slot is free
                # (a blocked matmul stalls the in-order PE stream)
                for _ in range(n):
                    feed_q.append({"f": lambda: None, "fin": False})

            def feed_oproj(qt):
                st8 = {}
                def alloc():
                    if "pp" not in st8:
                        st8["pp"] = ps_proj.tile(
                            [128, C], F32, tag="pp", name=f"fo{qt}")
                    return st8["pp"]
                for cc in range(2):
                    for nn in range(2):
                        def mm(cc=cc, nn=nn):
                            nc.tensor.matmul(
                                alloc()[:, bass.ts(nn, C // 2)],
                                oT[cc][:, bass.ts(qt, 128)],
                                wp[:, cc, bass.ts(nn, C // 2)],
                                start=(cc == 0), stop=(cc == 1))
                        feed_q.append({"f": mm, "fin": False})
                def fin():
                    oproj_evac(st8["pp"], qt, "dve")
                feed_q.append({"f": fin, "fin": True})
                feed_nops(2)

            def pump(kt, n, block_fins=False):
                for _ in range(n):
                    if not feed_q:
                        return
                    if feed_q[0]["fin"] and block_fins:
                        return
                    feed_q.pop(0)["f"]()

            # ---- prologue: only what window (0,0) needs before its kt10:
            # kT(pair0), qT(block0, pair0), and all of V.  Everything else
            # flows through the feed queue: qT(mt1,b0) / kT(pair1) / the
            # block-1 q tiles chain one-at-a-time through the single
            # ps_proj slot (~13 kt each: 8 kt of matmuls + rope reads),
            # so the chain starts in the DMA-shadowed prologue (PE is
            # half idle there) via explicit pump calls, sized so only
            # q10's entries drain before the window loop.
            feed_qk(wq, qT, 1, 0, tail_nops=20)
            feed_qk(wk, kT, 1, 0, tail_nops=12)
            feed_qk(wk, kT, 1, 1, tail_nops=16)
            # k00, q00 and q10's fed matmuls all pace with the per-chunk
            # xa arrivals — they fill the DMA shadow; v and k01 follow.
            # The window loop starts right after v15, gated by q00's rope.
            qk_tile(wk, kT, 0, 0)
            qk_tile(wq, qT, 0, 0)
            pump(-1, 16)
            for lt in range(12):
                v_tile(lt)
            qk_tile(wk, kT, 0, 1)
            pump(-1, 8)
            for lt in range(12, nKT):
                v_tile(lt)
                pump(-1, 2)

            # remaining per-window feeder registration: q01 during (0,1)
            # (consumed from (1,0)), q11 during (0,2) (consumed (1,2)),
            # qb0's out-proj spread 2 tiles/window over (1,0)-(1,3).
            feeders = {
                (0, 1): [(feed_qk, (wq, qT, 0, 1))],
                (0, 2): [(feed_qk, (wq, qT, 1, 1))],
                (1, 0): [(feed_oproj, (qt,)) for qt in range(2)],
                (1, 1): [(feed_oproj, (qt,)) for qt in range(2, 4)],
                (1, 2): [(feed_oproj, (qt,)) for qt in range(4, 6)],
                (1, 3): [(feed_oproj, (qt,)) for qt in range(6, 8)],
            }

            def norm_head_sp(pos_c, mt_c, pr_c, qs_c, si):
                # softmax denominator sits replicated on partitions 0-63
                # (ones-columns of v_aug); values on 64-127.  All-DVE:
                # plain copy to SBUF (custom ops need base-0 SBUF input),
                # fast reciprocal, multiply — no cross-engine hop.
                sp = SPL[si]
                po = pos_c[si]
                dn = npool.tile([64, PW], F32, tag="dn")
                nc.vector.tensor_copy(dn[:], po[0:64, :])
                rc = npool.tile([64, PW], F32, tag="rc")
                nc.vector.reciprocal_approx_fast(rc[:], dn[:])
                nc.vector.tensor_tensor(
                    oT[mt_c][pr_c, qs_c][:, sp], po[64:128, :], rc[:], MULT)

            def norm_head(pos_c, mt_c, pr_c, qs_c):
                for si in range(len(SPL)):
                    norm_head_sp(pos_c, mt_c, pr_c, qs_c, si)

            def pv_issue(pos_w, h_w, pts_w, ktc, last):
                for si, sp in enumerate(SPL):
                    nc.tensor.matmul(
                        pos_w[si][:], v_aug[:, ktc, h_w, :],
                        pts_w[ktc][:, sp],
                        start=(ktc == 0), stop=(ktc == nKT - 1 and last))

            # ---- attention: exps never pause; PV trails by TRAIL k-tiles
            # and each window's trailing PVs + norm flush early in the next
            carry = None
            for qb in range(nNB):
                qs = bass.ds(qb * NB, NB)
                for h in range(H_LOC):
                    mt, hh = divmod(h, 2)
                    pr = slice(64 * hh, 64 * hh + 64)
                    for fn, args in feeders.get((qb, h), []):
                        fn(*args)
                    pos = [ps_o.tile([128, PW], F32, tag="po",
                                     name=f"po{qb}{h}{si}")
                           for si in range(len(SPL))]
                    pts = {}
                    for kt in range(nKT):
                        st = ps_big.tile([128, NB], F32, tag="big")
                        for sp in SPL:
                            nc.tensor.matmul(
                                st[:, sp], kT[mt][pr, bass.ts(kt, 128)],
                                qT[mt][pr, qs][:, sp],
                                start=True, stop=True)
                        pt = ptpool.tile([128, NB], BF, tag="pt")
                        nc.scalar.activation(
                            pt[:], st[:], mybir.ActivationFunctionType.Exp,
                            scale=float(D) ** -0.5)
                        pts[kt] = pt
                        if carry is not None and 1 <= kt <= 6:
                            c_pos, c_h, c_pts, c_mt, c_pr, c_qs = carry
                            if kt <= 5:
                                for ktc in (nKT - TRAIL + 2 * kt - 2,
                                            nKT - TRAIL + 2 * kt - 1):
                                    pv_issue(c_pos, c_h, c_pts, ktc, True)
                            else:
                                norm_head(c_pos, c_mt, c_pr, c_qs)
                                carry = None
                        # last window trails less so its PVs mostly finish
                        # inside the window (shorter serial tail)
                        trail_w = 4 if (qb, h) == (nNB - 1, H_LOC - 1) else TRAIL
                        if kt >= trail_w:
                            pv_issue(pos, h, pts, kt - trail_w, False)
                        # (1,0) feeders read qb0's oT, whose last head's
                        # norm is carry-flushed at kt6 — no pumping before
                        # that (a blocked matmul stalls the in-order PE).
                        # While a carry is pending, hold rope/evac fins so
                        # the carried norm's DVE ops aren't queued behind a
                        # 5-op rope chain (pos recycling gates next PV).
                        if (qb, h) != (1, 0) or kt >= 7:
                            pump(kt, 2, block_fins=(carry is not None))
                    carry = (pos, h, pts, mt, pr, qs)
            # final window: flush trailing PVs si-major so norm(sp0) issues
            # while si1's PVs still stream on the PE; interleave the two
            # norm halves with the out-proj tiles they gate (qt8-11 read
            # only sp0 columns of oT, qt12-15 only sp1), and spread the
            # evacuations over ACT and DVE so the tail isn't one-engine
            # paced.
            c_pos, c_h, c_pts, c_mt, c_pr, c_qs = carry
            for si, sp in enumerate(SPL):
                for ktc in range(nKT - 4, nKT):
                    nc.tensor.matmul(
                        c_pos[si][:], v_aug[:, ktc, c_h, :],
                        c_pts[ktc][:, sp],
                        start=False, stop=(ktc == nKT - 1))
                if si == 0:
                    norm_head_sp(c_pos, c_mt, c_pr, c_qs, 0)
            while feed_q:
                feed_q.pop(0)["f"]()

            # tail: q-block 1's out-proj, ping-pong proj/score PSUM pools,
            # evacuations alternating ACT/DVE so neither engine paces it
            for j in range(NB // 128):
                qt = NB // 128 + j
                pp = (ps_proj if j % 2 == 0 else ps_big).tile(
                    [128, C], F32, tag="pp" if j % 2 == 0 else "big",
                    name=f"tp{qt}")
                oproj_mms(pp, qt)
                oproj_evac(pp, qt, "act" if j % 2 == 0 else "dve")
                if j == 3:
                    norm_head_sp(c_pos, c_mt, c_pr, c_qs, 1)

    nc.compile()
    return nc


_CACHE = {}


def _get_program(L=L_FULL):
    if L not in _CACHE:
        _CACHE[L] = build_program(L)
    return _CACHE[L]


def make_core_inputs(x, w_qkv, b_qkv, w_proj, cos, sin, L=L_FULL):
    """Host-side shard/transpose/pad/cast. Returns in_maps for the 8 cores."""
    x = np.asarray(x, np.float32)
    w_qkv = np.asarray(w_qkv, np.float32)
    b_qkv = np.asarray(b_qkv, np.float32)
    w_proj = np.asarray(w_proj, np.float32)
    cos = np.asarray(cos, np.float32)
    sin = np.asarray(sin, np.float32)

    # replicated rope tables: [128, L] (2 heads stacked), sign folded into sin
    cT = cos.T.astype(np.float32)                      # [64, L]
    sT = sin.T.astype(np.float32)
    cosT = np.concatenate([cT, cT], 0)                 # [128, L]
    sx = np.concatenate([-sT[0:32], sT[32:64]], 0)
    sinX = np.concatenate([sx, sx], 0)
    cs = np.ascontiguousarray(
        np.stack([cosT, sinX], axis=1)).astype(BF16)   # [128, 2, L]

    def pack_k(mat_t):
        # mat_t: [C, M] (already transposed) -> [128, KCH, M] bf16
        Cdim, M = mat_t.shape
        assert Cdim == KCH * 128
        return np.ascontiguousarray(
            mat_t.reshape(KCH, 128, M).transpose(1, 0, 2)).astype(BF16)

    in_maps = []
    for c in range(N_CORES):
        b, hg = divmod(c, 4)
        h0 = H_LOC * hg
        r = slice(h0 * D, (h0 + H_LOC) * D)            # head-channel rows
        # per-partition q/k biases for the [d, L] layout (cols: q-mt0,
        # q-mt1, k-mt0, k-mt1); v bias is applied on the host
        bq, bk = b_qkv[r], b_qkv[C:][r]
        bqk = np.stack([bq[:128], bq[128:], bk[:128], bk[128:]], 1)
        in_maps.append({
            "xa": np.ascontiguousarray(
                x[b].T.reshape(KCH, 128, L)).astype(BF16),
            "wq": pack_k(w_qkv[r].T),
            "wk": pack_k(w_qkv[C:][r].T),
            "wv": pack_k(w_qkv[2 * C:][r].T),
            "wp": np.ascontiguousarray(
                w_proj[:, r].T.reshape(2, 128, C).transpose(1, 0, 2)).astype(BF16),
            "bb": np.ascontiguousarray(bqk, np.float32),
            "cs": cs,
        })
    return in_maps


def kernel(x, w_qkv, b_qkv, w_proj, cos, sin, mask=None, trace=False):
    nc = _get_program()
    in_maps = make_core_inputs(x, w_qkv, b_qkv, w_proj, cos, sin)
    res = bass_utils.run_bass_kernel_spmd(
        nc, in_maps, core_ids=list(range(N_CORES)), trace=trace)
    # v-bias enters the output linearly: + b_v @ w_proj.T on every row
    bias_row = (np.asarray(b_qkv, np.float32)[2 * C:]
                @ np.asarray(w_proj, np.float32).T)
    out = np.zeros((B, L_FULL, C), np.float32)
    for c in range(N_CORES):
        out[c // 4] += res.results[c]["o"].astype(np.float32).reshape(L_FULL, C)
    out += bias_row[None, None, :]
    if trace:
        kernel.last_results = res
    return out

